# revision 57
# baseline (speedup 1.0000x reference)
"""Distributed MQA attention block (N=2, L=1024, D=4544, H=71, Dkv=64) on 8 TRN2 cores.

Sharding: 2 batch groups x 4-way head tensor-parallel.
  core c = 4*g + j: batch g, heads [18j, 18j+18) (core j=3: 17 real heads + 1 zero pad).

v4 layout (vs v2):
  - ALL attention (both q-halves) is interleaved into the projection pair
    loop, so softmax exp (ScalarE) always hides under the PE-heavy
    projection stream and the PE never starves on exp results.
  - Scores matmuls are K=64 row-packed: head h0 runs on PE row-group (0,0),
    h1 on (64,0) concurrently -- half the scores cycles.
  - AV chains and exp skip causally-dead columns (c0 trim).
  - The AllGather is split into 3 segments (pairs 0-3 / 4-6 / 7-8) fired as
    the pair loop passes them, so every segment lands before dense needs it.
  - ag_in is pair-major ([qh0p0|qh0p1|qh1p0|qh1p1] x 64 rows per pair), and
    the dense contraction runs pair-major (t-major over (t, r)) with
    host-side row-permuted W_dense, so all gather addresses are
    core-independent (SPMD) and early AG segments feed the first chains.
"""

import sys

if "/opt/trn_rl_repo" not in sys.path:
    sys.path.insert(0, "/opt/trn_rl_repo")

import numpy as np
import ml_dtypes

import concourse.bass as bass
import concourse.bacc as bacc
import concourse.mybir as mybir
import concourse.tile as tile
from concourse.bass_utils import run_bass_kernel_spmd

BF16 = mybir.dt.bfloat16
F32 = mybir.dt.float32
AF = mybir.ActivationFunctionType

N, L, D = 2, 1024, 4544
H, DKV = 71, 64
NCORES, GSZ = 8, 4
HPC = 18                 # heads per core (last core of each group: 17 real + 1 pad)
DLOC = HPC * DKV         # 1152
DPAD = GSZ * DLOC        # 4608 = 36 * 128
ESH = D // GSZ           # 1136 output-column shard
NET = 36                 # e-contraction tiles over D=4544 (35 x 128 + 64 zero-pad)
NMT = DLOC // 128        # 9 m-tiles of Q^T rows (2 heads each)
QB = 512                 # q-block (half of L)
SCALE = 1.0 / np.sqrt(DKV)
REPLICA_GROUPS = [[0, 1, 2, 3], [4, 5, 6, 7]]

# AG segments in ag_in row space (pair hp occupies rows [hp*256, hp*256+256)
# as [qh0p0|qh0p1|qh1p0|qh1p1] x 64): one AllGather per pair for pairs 0-7,
# pipelined on the collective stream right behind the pair loop; pair 8 is
# split by q-half and fired as each half's norms land, so the very last
# gathers arrive long before the dense tail reaches them
SEGS = [(p * 256, (p + 1) * 256) for p in range(8)] + [(2176, 2304),
                                                       (2048, 2176)]

# dense chain order: pair-major over (t, r) -> global k-tile 9r+t
CHAIN = [(t, r) for t in range(NMT) for r in range(GSZ)]

_CACHE = {}


def _emit(tc, nc, io):
    xT, wq3, wdT, cosT, sinT, rot, eye, masks, ones64, out = (
        io["xT"], io["wq3"], io["wdT"], io["cosT"], io["sinT"], io["rot"],
        io["eye"], io["masks"], io["ones64"], io["out"],
    )

    # ---- persistent SBUF (live through the whole kernel) ----
    pers = tc.alloc_tile_pool(name="pers", bufs=1)
    qsb = pers.tile([128, NMT * 1024], BF16, tag="qsb")    # roped Q^T, 2 heads/tile
    ksbA = pers.tile([128, 1024], BF16, tag="ksbA")        # [K^T; 0]
    ksbB = pers.tile([128, 1024], BF16, tag="ksbB")        # [0; K^T]
    vsb = pers.tile([128, 8 * 65], BF16, tag="vsb")        # V [tok,64]+ones col
    cossb = pers.tile([128, 1024], BF16, tag="cossb")
    sinsb = pers.tile([128, 1024], BF16, tag="sinsb")
    rotsb = pers.tile([128, 128], BF16, tag="rotsb")
    eyesb = pers.tile([128, 128], BF16, tag="eyesb")
    masksb = pers.tile([128, 128], BF16, tag="masksb")
    onesb = pers.tile([1, 64], BF16, tag="onesb")          # ones lhsT for 1/sum bcast

    # first 12 W_dense k-tiles load during the projection phase (fits SBUF
    # alongside x^T); the rest stream in once x^T frees
    NWA = 12
    wdpa = tc.alloc_tile_pool(name="wdpa", bufs=1)
    wdsbA = wdpa.tile([128, NWA * ESH], BF16, tag="wdsbA")

    # ---- DRAM bounce buffers for the segmented AllGather ----
    # ag_in row block for pair hp: [hp*256 + qh*128 + par*64, +64)
    dram = tc.alloc_tile_pool(name="dram", bufs=1, space="DRAM")
    ag_in = dram.tile([NMT * 256, QB], BF16, tag="agin", name="agin")
    ag_out = [dram.tile([GSZ * (hi - lo), QB], BF16, tag=f"agout{s}",
                        name=f"agout{s}") for s, (lo, hi) in enumerate(SEGS)]

    # =========== Phase AB: projections + interleaved attention ===========
    # PSUM budget (8 banks): qps 2 + scores(sc, 2 bufs) 4 + AV/qrot(ac) 2.
    # Pool releases are LIFO per (space, side).
    xp = tc.alloc_tile_pool(name="xp", bufs=1, side="right")
    wqp = tc.alloc_tile_pool(name="wqp", bufs=3, side="right")
    ra = tc.alloc_tile_pool(name="ra", bufs=2, side="right")
    rt = tc.alloc_tile_pool(name="rt", bufs=1, side="right")
    ex = tc.alloc_tile_pool(name="ex", bufs=15)
    nr = tc.alloc_tile_pool(name="nr", bufs=1)
    ps = tc.alloc_tile_pool(name="ps", bufs=1, space="PSUM")    # qps slot

    def load_w_cols(dst, m):
        # host-pretiled weights: m-th [128, NET*128] slab is contiguous
        # (k-tile i at cols i*128, tail rows already zero-padded)
        nc.sync.dma_start(dst[:, :], wq3[:, m * NET * 128:(m + 1) * NET * 128])

    sc = tc.alloc_tile_pool(name="sc", bufs=2, space="PSUM", side="right")
    ac = tc.alloc_tile_pool(name="ac", bufs=2, space="PSUM", side="right")

    # ---- startup DMAs: KV + pair-0 weights chunked and interleaved with the
    # first x^T tiles, so the first k-tile matmuls start ~2us in ----
    def load_w_chunk(dst, m, ch, nch):
        step = NET // nch * 128
        nc.sync.dma_start(dst[:, ch * step:(ch + 1) * step],
                          wq3[:, m * NET * 128 + ch * step:
                              m * NET * 128 + (ch + 1) * step])

    wkv = wqp.tile([128, NET * 128], BF16, tag="wm")
    wqm0 = wqp.tile([128, NET * 128], BF16, tag="wm")
    xsb = xp.tile([128, NET * 1024], BF16, tag="xsb")
    nc.vector.memset(xsb[64:128, (NET - 1) * 1024:NET * 1024], 0.0)
    for ch in range(4):
        load_w_chunk(wkv, NMT, ch, 4)
        load_w_chunk(wqm0, 0, ch, 4)
        nc.sync.dma_start(xsb[:, ch * 1024:(ch + 1) * 1024],
                          xT[ch * 128:(ch + 1) * 128, :])
    for i in range(4, NET - 1):
        nc.sync.dma_start(xsb[:, i * 1024:(i + 1) * 1024],
                          xT[i * 128:(i + 1) * 128, :])
        if i == 4:
            nc.sync.dma_start(cossb[:, :], cosT[:, :])
            nc.sync.dma_start(sinsb[:, :], sinT[:, :])
            nc.sync.dma_start(rotsb[:, :], rot[:, :])
            nc.sync.dma_start(eyesb[:, :], eye[:, :])
            nc.sync.dma_start(masksb[:, :], masks[:, :])
            nc.sync.dma_start(onesb[:, :], ones64[:, :])
            nc.vector.memset(vsb[:, :], 1.0)
            nc.vector.memset(ksbA[:, :], 0.0)
            nc.vector.memset(ksbB[:, :], 0.0)
    nc.sync.dma_start(xsb[0:64, (NET - 1) * 1024:NET * 1024],
                      xT[(NET - 1) * 128:D, :])
    for i in range(NWA):
        nc.sync.dma_start(wdsbA[:, i * ESH:(i + 1) * ESH],
                          wdT[i * 128:(i + 1) * 128, :])

    # ---- K/V + pair-0 Q projections interleaved: the first pass over x^T
    # is DMA-arrival-paced, so the pair-0 chain rides along for free.
    # The projection PSUM is split into two per-half tags (qp0/qp1) so a
    # pair's q-half-0 bank frees as soon as its copy lands -- the next
    # pair's pass-0 never waits on this pair's pass-1 copy. ----
    kvps = [ps.tile([128, QB], F32, tag=f"qp{q}", name=f"kv{q}")
            for q in range(2)]
    qps0 = sc.tile([128, 1024], F32, tag="sc", name="qps0")
    for i in range(NET):
        for q in range(2):
            nc.tensor.matmul(
                kvps[q][:, :],
                lhsT=wkv[:, i * 128:(i + 1) * 128],
                rhs=xsb[:, i * 1024 + q * QB:i * 1024 + (q + 1) * QB],
                start=(i == 0), stop=(i == NET - 1),
            )
        for q in range(2):
            nc.tensor.matmul(
                qps0[:, q * QB:(q + 1) * QB],
                lhsT=wqm0[:, i * 128:(i + 1) * 128],
                rhs=xsb[:, i * 1024 + q * QB:i * 1024 + (q + 1) * QB],
                start=(i == 0), stop=(i == NET - 1),
            )
    kvraw = ra.tile([128, 1024], BF16, tag="ra")
    for q in range(2):
        nc.vector.tensor_copy(kvraw[:, q * QB:(q + 1) * QB], kvps[q][:, :])
    qraw0 = ra.tile([128, 1024], BF16, tag="ra")
    nc.vector.tensor_copy(qraw0[:, :], qps0[:, :])
    # rope K (rows 0:64) into ksbA[0:64], then copy into ksbB[64:128]
    krot = [ps.tile([128, QB], F32, tag=f"qp{q}", name=f"kr{q}")
            for q in range(2)]
    for q in range(2):
        nc.tensor.matmul(krot[q][0:64, :],
                         lhsT=rotsb[0:64, 0:64],
                         rhs=kvraw[0:64, q * QB:(q + 1) * QB],
                         start=True, stop=True)
    for q in range(2):
        s = slice(q * QB, (q + 1) * QB)
        t1 = rt.tile([128, QB], F32, tag="t1")
        t2 = rt.tile([128, QB], F32, tag="t2")
        nc.vector.tensor_mul(t1[0:64, :], kvraw[0:64, s], cossb[0:64, s])
        nc.vector.tensor_mul(t2[0:64, :], krot[q][0:64, :], sinsb[0:64, s])
        nc.vector.tensor_add(ksbA[0:64, s], t1[0:64, :], t2[0:64, :])
    nc.sync.dma_start(ksbB[64:128, :], ksbA[0:64, :])

    # V^T (kvraw rows 64:128) -> transpose to V [tok, 64] chunks in vsb,
    # rotating through the (otherwise idle) qp0 slot
    for t8 in range(8):
        vtp = ps.tile([128, 64], BF16, tag="qp0", name=f"vtp{t8}")
        nc.tensor.transpose(vtp[:, :],
                            kvraw[64:128, t8 * 128:(t8 + 1) * 128],
                            eyesb[64:128, 64:128])
        nc.scalar.copy(vsb[:, t8 * 65:t8 * 65 + 64], vtp[:, :])

    def s_tile(qh, hp, kt):
        # scores + exp + mask for one k-tile (both heads). K=64 row-packed:
        # head h0 streams through PE row-group (0,0), h1 through (64,0) --
        # the two matmuls run concurrently on disjoint 64-row halves of the
        # array. Diagonal tile 4*qh+j: columns below 128*j fully masked
        # (never read downstream), the [128j, 128j+128) block gets the
        # triangular mask.
        var = kt - 4 * qh
        c0 = 128 * var if var > 0 else 0
        scp = sc.tile([128, 1024], F32, tag="sc")
        nc.tensor.matmul(
            scp[:, c0:QB],
            lhsT=ksbA[0:64, kt * 128:(kt + 1) * 128],
            rhs=qsb[0:64, hp * 1024 + qh * QB + c0:hp * 1024 + (qh + 1) * QB],
            start=True, stop=True,
        )
        nc.tensor.matmul(
            scp[:, QB + c0:2 * QB],
            lhsT=ksbB[64:128, kt * 128:(kt + 1) * 128],
            rhs=qsb[64:128, hp * 1024 + qh * QB + c0:hp * 1024 + (qh + 1) * QB],
            start=True, stop=True,
        )
        # exp split per head half: each AV chain waits only on its own
        # half's exp, halving the scores->exp->AV pipeline link latency
        es = ex.tile([128, 1024], BF16, tag="es")
        for par in range(2):
            nc.scalar.activation(es[:, par * QB + c0:(par + 1) * QB],
                                 scp[:, par * QB + c0:(par + 1) * QB],
                                 AF.Exp, scale=SCALE)
        if var >= 0:
            for par in range(2):
                nc.vector.tensor_mul(
                    es[:, par * QB + c0:par * QB + c0 + 128],
                    es[:, par * QB + c0:par * QB + c0 + 128],
                    masksb[:, 0:128])
        return es

    def av_chain(qh, par, ess, acc, kts):
        # causal trim: es tile kt is zero (masked) for local queries < c0,
        # so the accumulating matmul skips those columns (kt=0 has c0=0 and
        # start=True initializes the full bank range).
        nkt = 4 * qh + 4
        for kt in kts:
            var = kt - 4 * qh
            c0 = 128 * var if var > 0 else 0
            nc.tensor.matmul(
                acc[0:65, c0:QB],
                lhsT=vsb[:, kt * 65:(kt + 1) * 65],
                rhs=ess[kt][:, par * QB + c0:(par + 1) * QB],
                start=(kt == 0), stop=(kt == nkt - 1),
            )

    def norm_pre(par, acc):
        # softmax 1/sum on VectorE; priority-boosted -- these three tiny ops
        # gate the rbp broadcast matmul, which otherwise stalls the PE in the
        # late pairs when the projection filler is exhausted
        with tc.high_priority():
            stg = nr.tile([1, QB], F32, tag=f"stg{par}", name=f"stg{par}")
            nc.vector.tensor_copy(stg[:, :], acc[64:65, :])
            rcf = nr.tile([1, QB], F32, tag=f"rcf{par}", name=f"rcf{par}")
            nc.vector.reciprocal_approx_fast(rcf[:, :], stg[:, :])
            rc = nr.tile([1, QB], BF16, tag=f"rc{par}", name=f"rc{par}")
            with nc.allow_low_precision(reason="softmax 1/sum in bf16"):
                nc.vector.tensor_copy(rc[:, :], rcf[:, :])
        return rc

    def norm_post(qh, hp, par, acc, rc):
        rbp = sc.tile([128, 1024], F32, tag="sc")
        nc.tensor.matmul(rbp[0:64, 0:QB], lhsT=onesb[0:1, :],
                         rhs=rc[0:1, :], start=True, stop=True)
        rbs = nr.tile([64, QB], BF16, tag=f"rbs{par}", name=f"rbs{par}")
        nc.vector.tensor_copy(rbs[:, :], rbp[0:64, 0:QB])
        asb = nr.tile([64, QB], BF16, tag=f"asb{par}", name=f"asb{par}")
        nc.vector.tensor_mul(asb[:, :], acc[0:64, :], rbs[:, :])
        # SWDGE (gpsimd) DMA: completion semaphores separate from the shared
        # HWDGE queues, so the AllGather trigger thresholds only count these
        row = hp * 256 + qh * 128 + par * 64
        nc.gpsimd.dma_start(ag_in[row:row + 64, :], asb[:, :])

    def attn(qh, hp):
        nkt = 4 * qh + 4
        ess = [s_tile(qh, hp, kt) for kt in range(nkt)]
        accs, rcs = [], []
        for par in range(2):
            acc = ac.tile([128, QB], F32, tag="ac")
            av_chain(qh, par, ess, acc, range(nkt))
            rcs.append(norm_pre(par, acc))
            accs.append(acc)
        for par in range(2):
            norm_post(qh, hp, par, accs[par], rcs[par])

    def rope_half(hp, q, qraw, qr):
        s = slice(q * QB, (q + 1) * QB)
        nc.tensor.matmul(qr[:, :], lhsT=rotsb[:, :], rhs=qraw[:, s],
                         start=True, stop=True)
        t1 = rt.tile([128, QB], F32, tag="t1")
        t2 = rt.tile([128, QB], F32, tag="t2")
        nc.vector.tensor_mul(t1[:, :], qraw[:, s], cossb[:, s])
        nc.vector.tensor_mul(t2[:, :], qr[:, :], sinsb[:, s])
        nc.vector.tensor_add(qsb[:, hp * 1024 + q * QB:hp * 1024 + (q + 1) * QB],
                             t1[:, :], t2[:, :])

    def fire_ag(seg):
        lo, hi = SEGS[seg]
        nc.gpsimd.collective_compute(
            "AllGather", mybir.AluOpType.bypass,
            ins=[ag_in[lo:hi, :].opt()],
            outs=[ag_out[seg].opt()],
            replica_groups=REPLICA_GROUPS,
        )

    # ---- pair-0 RoPE + attention (its projection rode the KV window) ----
    qrots0 = [ac.tile([128, QB], F32, tag="ac", name=f"qrot0{q}")
              for q in range(2)]
    rope_half(0, 0, qraw0, qrots0[0])
    rope_half(0, 1, qraw0, qrots0[1])
    attn(1, 0)
    attn(0, 0)
    fire_ag(0)

    # ---- Q projection + RoPE + full attention, per m-tile pair ----
    for hp in range(1, NMT):
        wqm = wqp.tile([128, NET * 128], BF16, tag="wm")
        load_w_cols(wqm, hp)
        qps = [ps.tile([128, QB], F32, tag=f"qp{q}", name=f"qps{q}")
               for q in range(2)]
        qraw = ra.tile([128, 1024], BF16, tag="ra")
        qrots = [ac.tile([128, QB], F32, tag="ac", name=f"qrot{q}")
                 for q in range(2)]
        for i in range(NET):
            nc.tensor.matmul(
                qps[0][:, :], lhsT=wqm[:, i * 128:(i + 1) * 128],
                rhs=xsb[:, i * 1024:i * 1024 + QB],
                start=(i == 0), stop=(i == NET - 1),
            )
        with tc.high_priority():
            nc.vector.tensor_copy(qraw[:, 0:QB], qps[0][:, :])
        for i in range(NET):
            nc.tensor.matmul(
                qps[1][:, :], lhsT=wqm[:, i * 128:(i + 1) * 128],
                rhs=xsb[:, i * 1024 + QB:i * 1024 + 2 * QB],
                start=(i == 0), stop=(i == NET - 1),
            )
            if i == 5:
                rope_half(hp, 0, qraw, qrots[0])
        # only the PSUM->SBUF copies are priority-boosted (they gate the rope
        # -> scores chain and must not queue behind bulk DVE/ACT work); PE
        # attention ops keep natural order so the static schedule interleaves
        # them with the next pair's projection stream
        with tc.high_priority():
            nc.vector.tensor_copy(qraw[:, QB:2 * QB], qps[1][:, :])
        rope_half(hp, 1, qraw, qrots[1])
        attn(1, hp)
        if hp == NMT - 1:
            fire_ag(8)      # pair-8 q-half-1 rows, as soon as they land
        attn(0, hp)
        fire_ag(9 if hp == NMT - 1 else hp)
        if hp == NMT - 2:
            # scheduler fence BEFORE the last pair: dense work may interleave
            # with pair-8's attention tail, but cannot be hoisted ahead of the
            # earlier projection stream / AG triggers
            tc.no_sync_barrier()

    # free x^T/W_q SBUF; W_dense loads stream in under the attention tail
    rt.release()
    ra.release()
    wqp.release()
    xp.release()
    wdp = tc.alloc_tile_pool(name="wdp", bufs=1, side="right")
    wdsbB = wdp.tile([128, (NET - NWA) * ESH], BF16, tag="wdsbB")
    for i in range(NET - NWA):
        # scalar (ACT) HWDGE ring: keeps these 7MB of loads off the sync
        # ring so the dense gather DMAs are not queued behind them
        nc.scalar.dma_start(wdsbB[:, i * ESH:(i + 1) * ESH],
                            wdT[(NWA + i) * 128:(NWA + i + 1) * 128, :])

    # =========== dense: out^T[e_shard, q] = W_d^T[dpad, e].T @ attn^T ===========
    # wdT rows are host-permuted to chain order (pair-major over (t, r)),
    # so chain position p contracts global k-tile 9r+t.
    # The projection PSUM (ps) frees as soon as pair-8's projection is copied
    # out, so two dense chains (dpe) can run in the ACT-bound attention tail;
    # the remaining six (dp) start once the attention PSUM pools release.
    ps.release()
    dpe = tc.alloc_tile_pool(name="dpe", bufs=1, space="PSUM")
    gp0 = tc.alloc_tile_pool(name="gp0", bufs=1, side="right")
    op = tc.alloc_tile_pool(name="op", bufs=2, side="right")

    def gather_src(qh, t, r):
        row = t * 256 + qh * 128
        seg = next(s for s, (lo, hi) in enumerate(SEGS) if lo <= row < hi)
        lo, hi = SEGS[seg]
        srow = r * (hi - lo) + (row - lo)
        return ag_out[seg][srow:srow + 128, :]

    # qh0's gather buffer fits alongside the still-live es pool; qh1's is
    # allocated after the attention pools release
    gath = [gp0.tile([128, NET * QB], BF16, tag="gath0", name="gath0"), None]

    def emit_gathers(qh):
        # per-tile gather DMAs in chain order: chain MMs wait only on their
        # own tile, so the i-minor chains start on the first landed tile
        for p, (t, r) in enumerate(CHAIN):
            nc.sync.dma_start(gath[qh][:, p * QB:(p + 1) * QB],
                              gather_src(qh, t, r))

    emit_gathers(0)

    def dense_chain(dtile, qh, m, rows=128):
        for i in range(NET):
            wds, ii = (wdsbA, i) if i < NWA else (wdsbB, i - NWA)
            nc.tensor.matmul(
                dtile[0:rows, :],
                lhsT=wds[:, ii * ESH + m * 128:ii * ESH + m * 128 + rows],
                rhs=gath[qh][:, i * QB:(i + 1) * QB],
                start=(i == 0), stop=(i == NET - 1),
            )

    def evac(dtile, qh, m, rows=128):
        osb = op.tile([128, QB], F32, tag="op")
        nc.scalar.copy(osb[0:rows, :], dtile[0:rows, :])
        nc.sync.dma_start(out[m * 128:m * 128 + rows, qh * QB:(qh + 1) * QB],
                          osb[0:rows, :])

    # early chains m=0,1 of q-half 0 -- fill the attention-tail PE bubbles
    dpssE = [dpe.tile([128, QB], F32, tag=f"dpsE{m}", name=f"dpsE{m}")
             for m in range(2)]
    for m in range(2):
        dense_chain(dpssE[m], 0, m)

    ac.release()
    sc.release()
    nr.release()
    ex.release()
    dp = tc.alloc_tile_pool(name="dp", bufs=1, space="PSUM")
    gp1 = tc.alloc_tile_pool(name="gp1", bufs=1, side="right")
    gath[1] = gp1.tile([128, NET * QB], BF16, tag="gath1", name="gath1")
    emit_gathers(1)

    # i-minor dense: parallel accumulation chains (one PSUM bank each) so the
    # first gather tiles feed all chains and the DMA stays ahead of the PE
    for qh in range(2):
        if qh == 0:
            dpss = dpssE + [dp.tile([128, QB], F32, tag=f"dps{m}",
                                    name=f"dps{m}") for m in range(2, 8)]
            for m in range(2, 8):
                dense_chain(dpss[m], 0, m)
        else:
            dpss = [(dpe if m < 2 else dp).tile(
                [128, QB], F32, tag=f"dps{'E' if m < 2 else ''}{m % 8 if m >= 2 else m}",
                name=f"q1dps{m}") for m in range(8)]
            for m in range(8):
                dense_chain(dpss[m], 1, m)
        for m in range(8):
            evac(dpss[m], qh, m)
        dps = dp.tile([128, QB], F32, tag="dps2", name=f"rag{qh}")
        dense_chain(dps, qh, 8, rows=112)
        evac(dps, qh, 8, rows=112)

    dp.release()
    dpe.release()
    gp1.release()
    op.release()
    gp0.release()
    wdp.release()
    wdpa.release()
    pers.release()
    dram.release()


def build():
    if "nc" in _CACHE:
        return _CACHE["nc"]
    nc = bacc.Bacc("TRN2", target_bir_lowering=False, debug=False,
                   num_devices=NCORES)
    io = {
        "xT": nc.dram_tensor("xT", [D, L], BF16, kind="ExternalInput").ap(),
        "wq3": nc.dram_tensor("wq3", [128, (NMT + 1) * NET * 128], BF16,
                              kind="ExternalInput").ap(),
        "wdT": nc.dram_tensor("wdT", [DPAD, ESH], BF16, kind="ExternalInput").ap(),
        "cosT": nc.dram_tensor("cosT", [128, L], BF16, kind="ExternalInput").ap(),
        "sinT": nc.dram_tensor("sinT", [128, L], BF16, kind="ExternalInput").ap(),
        "rot": nc.dram_tensor("rot", [128, 128], BF16, kind="ExternalInput").ap(),
        "eye": nc.dram_tensor("eye", [128, 128], BF16, kind="ExternalInput").ap(),
        "masks": nc.dram_tensor("masks", [128, 128], BF16,
                                kind="ExternalInput").ap(),
        "ones64": nc.dram_tensor("ones64", [1, 64], BF16,
                                 kind="ExternalInput").ap(),
        "out": nc.dram_tensor("out", [ESH, L], F32, kind="ExternalOutput").ap(),
    }
    with tile.TileContext(nc) as tc:
        _emit(tc, nc, io)
    nc.compile()
    _CACHE["nc"] = nc
    return nc


def make_in_maps(hidden_states, W_qkv, W_dense):
    bf = ml_dtypes.bfloat16
    x = np.asarray(hidden_states, np.float32)
    Wqkv = np.asarray(W_qkv, np.float32)
    Wd = np.asarray(W_dense, np.float32)

    # rope tables, transposed [64, L], replicated to both 64-row halves
    inv = 1.0 / (10000.0 ** (np.arange(0, DKV, 2, dtype=np.float32) / DKV))
    t = np.arange(L, dtype=np.float32)
    freqs = np.outer(t, inv)
    emb = np.concatenate([freqs, freqs], axis=1)          # [L, 64]
    cosT = np.tile(np.cos(emb).T, (2, 1)).astype(bf)      # [128, L]
    sinT = np.tile(np.sin(emb).T, (2, 1)).astype(bf)

    # rotate_half as a matmul: qrot = R1 @ q; lhsT = R1^T; 2-head block diagonal
    R1 = np.zeros((DKV, DKV), np.float32)
    for i in range(32):
        R1[i, i + 32] = -1.0
        R1[i + 32, i] = 1.0
    R2 = np.zeros((128, 128), np.float32)
    R2[:64, :64] = R1
    R2[64:, 64:] = R1
    rot = R2.T.copy().astype(bf)

    eye = np.eye(128, dtype=np.float32).astype(bf)

    ones64 = np.ones((1, 64), np.float32).astype(bf)

    # triangular causal mask for the 128x128 diagonal block
    kk = np.arange(128)[:, None]
    qq = np.arange(128)[None, :]
    masks = (kk <= qq).astype(np.float32).astype(bf)

    # padded dense weights: W_d^T with 64 zero rows appended (pad head),
    # row-tiles permuted to the dense chain order (pair-major over (t, r))
    wdT_full = np.concatenate([Wd.T, np.zeros((DPAD - D, D), np.float32)], axis=0)
    wdT_full = wdT_full.reshape(NET, 128, D)
    perm = [NMT * r + t for (t, r) in CHAIN]
    wdT_full = np.ascontiguousarray(wdT_full[perm]).reshape(DPAD, D).astype(bf)

    in_maps = []
    for c in range(NCORES):
        gg, j = divmod(c, GSZ)
        h0 = HPC * j
        nh = HPC if j < GSZ - 1 else H - HPC * (GSZ - 1)  # 18,18,18,17
        WqT = np.zeros((D, DLOC), np.float32)
        WqT[:, :nh * DKV] = Wqkv[DKV * h0:DKV * (h0 + nh)].T
        # pretile [D, 1280] -> [128, 10*36*128]: slab m holds k-tile i at
        # cols (m*36+i)*128, rows zero-padded to 4608
        Wcat = np.concatenate([WqT, Wqkv[H * DKV:].T], axis=1)   # [D, 1280]
        Wp = np.zeros((NET * 128, (NMT + 1) * 128), np.float32)
        Wp[:D] = Wcat
        wq3 = np.ascontiguousarray(
            Wp.reshape(NET, 128, NMT + 1, 128).transpose(1, 2, 0, 3)
            .reshape(128, (NMT + 1) * NET * 128)).astype(bf)
        in_maps.append({
            "xT": np.ascontiguousarray(x[gg].T).astype(bf),
            "wq3": wq3,
            "wdT": np.ascontiguousarray(wdT_full[:, ESH * j:ESH * (j + 1)]),
            "cosT": cosT, "sinT": sinT, "rot": rot, "eye": eye,
            "masks": masks, "ones64": ones64,
        })
    return in_maps


def assemble(results):
    out = np.empty((N, L, D), np.float32)
    for c in range(NCORES):
        gg, j = divmod(c, GSZ)
        out[gg, :, ESH * j:ESH * (j + 1)] = results[c]["out"].T
    return out


def kernel(hidden_states, W_qkv, W_dense):
    nc = build()
    in_maps = make_in_maps(hidden_states, W_qkv, W_dense)
    res = run_bass_kernel_spmd(nc, in_maps, core_ids=list(range(NCORES)))
    return assemble(res.results)


if __name__ == "__main__":
    import reference
    inputs = reference.setup_inputs()
    out = kernel(**{k: np.asarray(v) for k, v in inputs.items()})
    print("out", out.shape, out.dtype)


# revision 59
# speedup vs baseline: 1.0330x; 1.0330x over previous
"""Distributed MQA attention block (N=2, L=1024, D=4544, H=71, Dkv=64) on 8 TRN2 cores.

Sharding: 2 batch groups x 4-way head tensor-parallel.
  core c = 4*g + j: batch g, heads [18j, 18j+18) (core j=3: 17 real heads + 1 zero pad).

v4 layout (vs v2):
  - ALL attention (both q-halves) is interleaved into the projection pair
    loop, so softmax exp (ScalarE) always hides under the PE-heavy
    projection stream and the PE never starves on exp results.
  - Scores matmuls are K=64 row-packed: head h0 runs on PE row-group (0,0),
    h1 on (64,0) concurrently -- half the scores cycles.
  - AV chains and exp skip causally-dead columns (c0 trim).
  - The AllGather is split into 3 segments (pairs 0-3 / 4-6 / 7-8) fired as
    the pair loop passes them, so every segment lands before dense needs it.
  - ag_in is pair-major ([qh0p0|qh0p1|qh1p0|qh1p1] x 64 rows per pair), and
    the dense contraction runs pair-major (t-major over (t, r)) with
    host-side row-permuted W_dense, so all gather addresses are
    core-independent (SPMD) and early AG segments feed the first chains.
"""

import sys

if "/opt/trn_rl_repo" not in sys.path:
    sys.path.insert(0, "/opt/trn_rl_repo")

import numpy as np
import ml_dtypes

import concourse.bass as bass
import concourse.bacc as bacc
import concourse.mybir as mybir
import concourse.tile as tile
from concourse.bass_utils import run_bass_kernel_spmd

BF16 = mybir.dt.bfloat16
F32 = mybir.dt.float32
AF = mybir.ActivationFunctionType

N, L, D = 2, 1024, 4544
H, DKV = 71, 64
NCORES, GSZ = 8, 4
HPC = 18                 # heads per core (last core of each group: 17 real + 1 pad)
DLOC = HPC * DKV         # 1152
DPAD = GSZ * DLOC        # 4608 = 36 * 128
ESH = D // GSZ           # 1136 output-column shard
NET = 36                 # e-contraction tiles over D=4544 (35 x 128 + 64 zero-pad)
NMT = DLOC // 128        # 9 m-tiles of Q^T rows (2 heads each)
QB = 512                 # q-block (half of L)
SCALE = 1.0 / np.sqrt(DKV)
REPLICA_GROUPS = [[0, 1, 2, 3], [4, 5, 6, 7]]

# AG segments in ag_in row space (pair hp occupies rows [hp*256, hp*256+256)
# as [qh0p0|qh0p1|qh1p0|qh1p1] x 64): one AllGather per pair for pairs 0-7,
# pipelined on the collective stream right behind the pair loop; pair 8 is
# split by q-half and fired as each half's norms land, so the very last
# gathers arrive long before the dense tail reaches them
SEGS = [(p * 256, (p + 1) * 256) for p in range(8)] + [(2176, 2304),
                                                       (2048, 2176)]

# dense chain order: pair-major over (t, r) -> global k-tile 9r+t
CHAIN = [(t, r) for t in range(NMT) for r in range(GSZ)]

_CACHE = {}


def _emit(tc, nc, io):
    xT, wq3, wdT, cosT, sinT, rot, eye, masks, ones64, out = (
        io["xT"], io["wq3"], io["wdT"], io["cosT"], io["sinT"], io["rot"],
        io["eye"], io["masks"], io["ones64"], io["out"],
    )

    # ---- persistent SBUF (live through the whole kernel) ----
    pers = tc.alloc_tile_pool(name="pers", bufs=1)
    qsb = pers.tile([128, NMT * 1024], BF16, tag="qsb")    # roped Q^T, 2 heads/tile
    ksbA = pers.tile([128, 1024], BF16, tag="ksbA")        # [K^T; 0]
    ksbB = pers.tile([128, 1024], BF16, tag="ksbB")        # [0; K^T]
    vsb = pers.tile([128, 8 * 65], BF16, tag="vsb")        # V [tok,64]+ones col
    cossb = pers.tile([128, 1024], BF16, tag="cossb")
    sinsb = pers.tile([128, 1024], BF16, tag="sinsb")
    rotsb = pers.tile([128, 128], BF16, tag="rotsb")
    eyesb = pers.tile([128, 128], BF16, tag="eyesb")
    masksb = pers.tile([128, 128], BF16, tag="masksb")
    onesb = pers.tile([1, 64], BF16, tag="onesb")          # ones lhsT for 1/sum bcast

    # first 12 W_dense k-tiles load during the projection phase (fits SBUF
    # alongside x^T); the rest stream in once x^T frees
    NWA = 12
    wdpa = tc.alloc_tile_pool(name="wdpa", bufs=1)
    wdsbA = wdpa.tile([128, NWA * ESH], BF16, tag="wdsbA")

    # ---- DRAM bounce buffers for the segmented AllGather ----
    # ag_in row block for pair hp: [hp*256 + qh*128 + par*64, +64)
    dram = tc.alloc_tile_pool(name="dram", bufs=1, space="DRAM")
    ag_in = dram.tile([NMT * 256, QB], BF16, tag="agin", name="agin")
    ag_out = [dram.tile([GSZ * (hi - lo), QB], BF16, tag=f"agout{s}",
                        name=f"agout{s}") for s, (lo, hi) in enumerate(SEGS)]

    # =========== Phase AB: projections + interleaved attention ===========
    # PSUM budget (8 banks): qps 2 + scores(sc, 2 bufs) 4 + AV/qrot(ac) 2.
    # Pool releases are LIFO per (space, side).
    xp = tc.alloc_tile_pool(name="xp", bufs=1, side="right")
    wqp = tc.alloc_tile_pool(name="wqp", bufs=3, side="right")
    ra = tc.alloc_tile_pool(name="ra", bufs=2, side="right")
    rt = tc.alloc_tile_pool(name="rt", bufs=1, side="right")
    ex = tc.alloc_tile_pool(name="ex", bufs=15)
    nr = tc.alloc_tile_pool(name="nr", bufs=1)
    ps = tc.alloc_tile_pool(name="ps", bufs=1, space="PSUM")    # qps slot

    def load_w_cols(dst, m):
        # host-pretiled weights: m-th [128, NET*128] slab is contiguous
        # (k-tile i at cols i*128, tail rows already zero-padded)
        nc.sync.dma_start(dst[:, :], wq3[:, m * NET * 128:(m + 1) * NET * 128])

    sc = tc.alloc_tile_pool(name="sc", bufs=2, space="PSUM", side="right")
    ac = tc.alloc_tile_pool(name="ac", bufs=2, space="PSUM", side="right")

    # ---- startup DMAs: KV + pair-0 weights chunked and interleaved with the
    # first x^T tiles, so the first k-tile matmuls start ~2us in ----
    def load_w_chunk(dst, m, ch, nch):
        step = NET // nch * 128
        nc.sync.dma_start(dst[:, ch * step:(ch + 1) * step],
                          wq3[:, m * NET * 128 + ch * step:
                              m * NET * 128 + (ch + 1) * step])

    wkv = wqp.tile([128, NET * 128], BF16, tag="wm")
    wqm0 = wqp.tile([128, NET * 128], BF16, tag="wm")
    xsb = xp.tile([128, NET * 1024], BF16, tag="xsb")
    nc.vector.memset(xsb[64:128, (NET - 1) * 1024:NET * 1024], 0.0)
    for ch in range(4):
        load_w_chunk(wkv, NMT, ch, 4)
        load_w_chunk(wqm0, 0, ch, 4)
        nc.sync.dma_start(xsb[:, ch * 1024:(ch + 1) * 1024],
                          xT[ch * 128:(ch + 1) * 128, :])
    for i in range(4, NET - 1):
        nc.sync.dma_start(xsb[:, i * 1024:(i + 1) * 1024],
                          xT[i * 128:(i + 1) * 128, :])
        if i == 4:
            nc.sync.dma_start(cossb[:, :], cosT[:, :])
            nc.sync.dma_start(sinsb[:, :], sinT[:, :])
            nc.sync.dma_start(rotsb[:, :], rot[:, :])
            nc.sync.dma_start(eyesb[:, :], eye[:, :])
            nc.sync.dma_start(masksb[:, :], masks[:, :])
            nc.sync.dma_start(onesb[:, :], ones64[:, :])
            nc.vector.memset(vsb[:, :], 1.0)
            nc.vector.memset(ksbA[:, :], 0.0)
            nc.vector.memset(ksbB[:, :], 0.0)
    nc.sync.dma_start(xsb[0:64, (NET - 1) * 1024:NET * 1024],
                      xT[(NET - 1) * 128:D, :])
    for i in range(NWA):
        nc.sync.dma_start(wdsbA[:, i * ESH:(i + 1) * ESH],
                          wdT[i * 128:(i + 1) * 128, :])

    # ---- K/V + pair-0 Q projections interleaved: the first pass over x^T
    # is DMA-arrival-paced, so the pair-0 chain rides along for free.
    # The projection PSUM is split into two per-half tags (qp0/qp1) so a
    # pair's q-half-0 bank frees as soon as its copy lands -- the next
    # pair's pass-0 never waits on this pair's pass-1 copy. ----
    kvps = [ps.tile([128, QB], F32, tag=f"qp{q}", name=f"kv{q}")
            for q in range(2)]
    qps0 = sc.tile([128, 1024], F32, tag="sc", name="qps0")
    for i in range(NET):
        for q in range(2):
            nc.tensor.matmul(
                kvps[q][:, :],
                lhsT=wkv[:, i * 128:(i + 1) * 128],
                rhs=xsb[:, i * 1024 + q * QB:i * 1024 + (q + 1) * QB],
                start=(i == 0), stop=(i == NET - 1),
            )
        for q in range(2):
            nc.tensor.matmul(
                qps0[:, q * QB:(q + 1) * QB],
                lhsT=wqm0[:, i * 128:(i + 1) * 128],
                rhs=xsb[:, i * 1024 + q * QB:i * 1024 + (q + 1) * QB],
                start=(i == 0), stop=(i == NET - 1),
            )
    kvraw = ra.tile([128, 1024], BF16, tag="ra")
    for q in range(2):
        nc.vector.tensor_copy(kvraw[:, q * QB:(q + 1) * QB], kvps[q][:, :])
    qraw0 = ra.tile([128, 1024], BF16, tag="ra")
    nc.vector.tensor_copy(qraw0[:, :], qps0[:, :])
    # rope K (rows 0:64) into ksbA[0:64], then copy into ksbB[64:128]
    krot = [ps.tile([128, QB], F32, tag=f"qp{q}", name=f"kr{q}")
            for q in range(2)]
    for q in range(2):
        nc.tensor.matmul(krot[q][0:64, :],
                         lhsT=rotsb[0:64, 0:64],
                         rhs=kvraw[0:64, q * QB:(q + 1) * QB],
                         start=True, stop=True)
    for q in range(2):
        s = slice(q * QB, (q + 1) * QB)
        t1 = rt.tile([128, QB], F32, tag="t1")
        t2 = rt.tile([128, QB], F32, tag="t2")
        nc.vector.tensor_mul(t1[0:64, :], kvraw[0:64, s], cossb[0:64, s])
        nc.vector.tensor_mul(t2[0:64, :], krot[q][0:64, :], sinsb[0:64, s])
        nc.vector.tensor_add(ksbA[0:64, s], t1[0:64, :], t2[0:64, :])
    nc.sync.dma_start(ksbB[64:128, :], ksbA[0:64, :])

    # V^T (kvraw rows 64:128) -> transpose to V [tok, 64] chunks in vsb,
    # rotating through the (otherwise idle) qp0 slot
    for t8 in range(8):
        vtp = ps.tile([128, 64], BF16, tag="qp0", name=f"vtp{t8}")
        nc.tensor.transpose(vtp[:, :],
                            kvraw[64:128, t8 * 128:(t8 + 1) * 128],
                            eyesb[64:128, 64:128])
        nc.scalar.copy(vsb[:, t8 * 65:t8 * 65 + 64], vtp[:, :])

    def s_tile(qh, hp, kt):
        # scores + exp + mask for one k-tile (both heads). K=64 row-packed:
        # head h0 streams through PE row-group (0,0), h1 through (64,0) --
        # the two matmuls run concurrently on disjoint 64-row halves of the
        # array. Diagonal tile 4*qh+j: columns below 128*j fully masked
        # (never read downstream), the [128j, 128j+128) block gets the
        # triangular mask.
        var = kt - 4 * qh
        c0 = 128 * var if var > 0 else 0
        scp = sc.tile([128, 1024], F32, tag="sc")
        nc.tensor.matmul(
            scp[:, c0:QB],
            lhsT=ksbA[0:64, kt * 128:(kt + 1) * 128],
            rhs=qsb[0:64, hp * 1024 + qh * QB + c0:hp * 1024 + (qh + 1) * QB],
            start=True, stop=True,
        )
        nc.tensor.matmul(
            scp[:, QB + c0:2 * QB],
            lhsT=ksbB[64:128, kt * 128:(kt + 1) * 128],
            rhs=qsb[64:128, hp * 1024 + qh * QB + c0:hp * 1024 + (qh + 1) * QB],
            start=True, stop=True,
        )
        es = ex.tile([128, 1024], BF16, tag="es")
        if c0 == 0:
            nc.scalar.activation(es[:, :], scp[:, :], AF.Exp, scale=SCALE)
        else:
            ev = es.rearrange("p (a b) -> p a b", a=2)
            sv = scp.rearrange("p (a b) -> p a b", a=2)
            nc.scalar.activation(ev[:, :, c0:QB], sv[:, :, c0:QB],
                                 AF.Exp, scale=SCALE)
        if var >= 0:
            for par in range(2):
                nc.vector.tensor_mul(
                    es[:, par * QB + c0:par * QB + c0 + 128],
                    es[:, par * QB + c0:par * QB + c0 + 128],
                    masksb[:, 0:128])
        return es

    def av_chain(qh, par, ess, acc, kts):
        # causal trim: es tile kt is zero (masked) for local queries < c0,
        # so the accumulating matmul skips those columns (kt=0 has c0=0 and
        # start=True initializes the full bank range).
        nkt = 4 * qh + 4
        for kt in kts:
            var = kt - 4 * qh
            c0 = 128 * var if var > 0 else 0
            nc.tensor.matmul(
                acc[0:65, c0:QB],
                lhsT=vsb[:, kt * 65:(kt + 1) * 65],
                rhs=ess[kt][:, par * QB + c0:(par + 1) * QB],
                start=(kt == 0), stop=(kt == nkt - 1),
            )

    def norm_pre(par, acc):
        # softmax 1/sum on VectorE; priority-boosted -- these three tiny ops
        # gate the rbp broadcast matmul, which otherwise stalls the PE in the
        # late pairs when the projection filler is exhausted
        with tc.high_priority():
            stg = nr.tile([1, QB], F32, tag=f"stg{par}", name=f"stg{par}")
            nc.vector.tensor_copy(stg[:, :], acc[64:65, :])
            rcf = nr.tile([1, QB], F32, tag=f"rcf{par}", name=f"rcf{par}")
            nc.vector.reciprocal_approx_fast(rcf[:, :], stg[:, :])
            rc = nr.tile([1, QB], BF16, tag=f"rc{par}", name=f"rc{par}")
            with nc.allow_low_precision(reason="softmax 1/sum in bf16"):
                nc.vector.tensor_copy(rc[:, :], rcf[:, :])
        return rc

    def norm_post(qh, hp, par, acc, rc):
        rbp = sc.tile([128, 1024], F32, tag="sc")
        nc.tensor.matmul(rbp[0:64, 0:QB], lhsT=onesb[0:1, :],
                         rhs=rc[0:1, :], start=True, stop=True)
        rbs = nr.tile([64, QB], BF16, tag=f"rbs{par}", name=f"rbs{par}")
        nc.vector.tensor_copy(rbs[:, :], rbp[0:64, 0:QB])
        asb = nr.tile([64, QB], BF16, tag=f"asb{par}", name=f"asb{par}")
        nc.vector.tensor_mul(asb[:, :], acc[0:64, :], rbs[:, :])
        # SWDGE (gpsimd) DMA: completion semaphores separate from the shared
        # HWDGE queues, so the AllGather trigger thresholds only count these
        row = hp * 256 + qh * 128 + par * 64
        nc.gpsimd.dma_start(ag_in[row:row + 64, :], asb[:, :])

    def attn(qh, hp):
        nkt = 4 * qh + 4
        ess = [s_tile(qh, hp, kt) for kt in range(nkt)]
        accs, rcs = [], []
        for par in range(2):
            acc = ac.tile([128, QB], F32, tag="ac")
            av_chain(qh, par, ess, acc, range(nkt))
            rcs.append(norm_pre(par, acc))
            accs.append(acc)
        for par in range(2):
            norm_post(qh, hp, par, accs[par], rcs[par])

    def rope_half(hp, q, qraw, qr):
        # the three DVE ops are priority-boosted: the final qsb add gates all
        # 24 scores matmuls of the pair, and must not queue behind bulk
        # mask-mul/norm DVE work in the late pairs
        s = slice(q * QB, (q + 1) * QB)
        nc.tensor.matmul(qr[:, :], lhsT=rotsb[:, :], rhs=qraw[:, s],
                         start=True, stop=True)
        with tc.high_priority():
            t1 = rt.tile([128, QB], F32, tag="t1")
            t2 = rt.tile([128, QB], F32, tag="t2")
            nc.vector.tensor_mul(t1[:, :], qraw[:, s], cossb[:, s])
            nc.vector.tensor_mul(t2[:, :], qr[:, :], sinsb[:, s])
            nc.vector.tensor_add(
                qsb[:, hp * 1024 + q * QB:hp * 1024 + (q + 1) * QB],
                t1[:, :], t2[:, :])

    def fire_ag(seg):
        lo, hi = SEGS[seg]
        nc.gpsimd.collective_compute(
            "AllGather", mybir.AluOpType.bypass,
            ins=[ag_in[lo:hi, :].opt()],
            outs=[ag_out[seg].opt()],
            replica_groups=REPLICA_GROUPS,
        )

    # ---- pair-0 RoPE + attention (its projection rode the KV window) ----
    qrots0 = [ac.tile([128, QB], F32, tag="ac", name=f"qrot0{q}")
              for q in range(2)]
    rope_half(0, 0, qraw0, qrots0[0])
    rope_half(0, 1, qraw0, qrots0[1])
    attn(1, 0)
    attn(0, 0)
    fire_ag(0)

    # ---- Q projection + RoPE + full attention, per m-tile pair ----
    for hp in range(1, NMT):
        wqm = wqp.tile([128, NET * 128], BF16, tag="wm")
        load_w_cols(wqm, hp)
        qps = [ps.tile([128, QB], F32, tag=f"qp{q}", name=f"qps{q}")
               for q in range(2)]
        qraw = ra.tile([128, 1024], BF16, tag="ra")
        qrots = [ac.tile([128, QB], F32, tag="ac", name=f"qrot{q}")
                 for q in range(2)]
        for i in range(NET):
            nc.tensor.matmul(
                qps[0][:, :], lhsT=wqm[:, i * 128:(i + 1) * 128],
                rhs=xsb[:, i * 1024:i * 1024 + QB],
                start=(i == 0), stop=(i == NET - 1),
            )
        with tc.high_priority():
            nc.vector.tensor_copy(qraw[:, 0:QB], qps[0][:, :])
        for i in range(NET):
            nc.tensor.matmul(
                qps[1][:, :], lhsT=wqm[:, i * 128:(i + 1) * 128],
                rhs=xsb[:, i * 1024 + QB:i * 1024 + 2 * QB],
                start=(i == 0), stop=(i == NET - 1),
            )
            if i == 5:
                rope_half(hp, 0, qraw, qrots[0])
        # only the PSUM->SBUF copies are priority-boosted (they gate the rope
        # -> scores chain and must not queue behind bulk DVE/ACT work); PE
        # attention ops keep natural order so the static schedule interleaves
        # them with the next pair's projection stream
        with tc.high_priority():
            nc.vector.tensor_copy(qraw[:, QB:2 * QB], qps[1][:, :])
        rope_half(hp, 1, qraw, qrots[1])
        attn(1, hp)
        if hp == NMT - 1:
            fire_ag(8)      # pair-8 q-half-1 rows, as soon as they land
        attn(0, hp)
        fire_ag(9 if hp == NMT - 1 else hp)
        if hp == NMT - 2:
            # scheduler fence BEFORE the last pair: dense work may interleave
            # with pair-8's attention tail, but cannot be hoisted ahead of the
            # earlier projection stream / AG triggers
            tc.no_sync_barrier()

    # free x^T/W_q SBUF; W_dense loads stream in under the attention tail
    rt.release()
    ra.release()
    wqp.release()
    xp.release()
    wdp = tc.alloc_tile_pool(name="wdp", bufs=1, side="right")
    wdsbB = wdp.tile([128, (NET - NWA) * ESH], BF16, tag="wdsbB")
    for i in range(NET - NWA):
        # scalar (ACT) HWDGE ring: keeps these 7MB of loads off the sync
        # ring so the dense gather DMAs are not queued behind them
        nc.scalar.dma_start(wdsbB[:, i * ESH:(i + 1) * ESH],
                            wdT[(NWA + i) * 128:(NWA + i + 1) * 128, :])

    # =========== dense: out^T[e_shard, q] = W_d^T[dpad, e].T @ attn^T ===========
    # wdT rows are host-permuted to chain order (pair-major over (t, r)),
    # so chain position p contracts global k-tile 9r+t.
    # The projection PSUM (ps) frees as soon as pair-8's projection is copied
    # out, so two dense chains (dpe) can run in the ACT-bound attention tail;
    # the remaining six (dp) start once the attention PSUM pools release.
    ps.release()
    dpe = tc.alloc_tile_pool(name="dpe", bufs=1, space="PSUM")
    gp0 = tc.alloc_tile_pool(name="gp0", bufs=1, side="right")
    op = tc.alloc_tile_pool(name="op", bufs=2, side="right")

    def gather_src(qh, t, r):
        row = t * 256 + qh * 128
        seg = next(s for s, (lo, hi) in enumerate(SEGS) if lo <= row < hi)
        lo, hi = SEGS[seg]
        srow = r * (hi - lo) + (row - lo)
        return ag_out[seg][srow:srow + 128, :]

    # qh0's gather buffer fits alongside the still-live es pool; qh1's is
    # allocated after the attention pools release
    gath = [gp0.tile([128, NET * QB], BF16, tag="gath0", name="gath0"), None]

    def emit_gathers(qh):
        # per-tile gather DMAs in chain order: chain MMs wait only on their
        # own tile, so the i-minor chains start on the first landed tile
        for p, (t, r) in enumerate(CHAIN):
            nc.sync.dma_start(gath[qh][:, p * QB:(p + 1) * QB],
                              gather_src(qh, t, r))

    emit_gathers(0)

    def dense_chain(dtile, qh, m, rows=128):
        for i in range(NET):
            wds, ii = (wdsbA, i) if i < NWA else (wdsbB, i - NWA)
            nc.tensor.matmul(
                dtile[0:rows, :],
                lhsT=wds[:, ii * ESH + m * 128:ii * ESH + m * 128 + rows],
                rhs=gath[qh][:, i * QB:(i + 1) * QB],
                start=(i == 0), stop=(i == NET - 1),
            )

    def evac(dtile, qh, m, rows=128):
        osb = op.tile([128, QB], F32, tag="op")
        nc.scalar.copy(osb[0:rows, :], dtile[0:rows, :])
        nc.sync.dma_start(out[m * 128:m * 128 + rows, qh * QB:(qh + 1) * QB],
                          osb[0:rows, :])

    # early chains m=0,1 of q-half 0 -- fill the attention-tail PE bubbles
    dpssE = [dpe.tile([128, QB], F32, tag=f"dpsE{m}", name=f"dpsE{m}")
             for m in range(2)]
    for m in range(2):
        dense_chain(dpssE[m], 0, m)

    ac.release()
    sc.release()
    nr.release()
    ex.release()
    dp = tc.alloc_tile_pool(name="dp", bufs=1, space="PSUM")
    gp1 = tc.alloc_tile_pool(name="gp1", bufs=1, side="right")
    gath[1] = gp1.tile([128, NET * QB], BF16, tag="gath1", name="gath1")
    emit_gathers(1)

    # i-minor dense: parallel accumulation chains (one PSUM bank each) so the
    # first gather tiles feed all chains and the DMA stays ahead of the PE
    for qh in range(2):
        if qh == 0:
            dpss = dpssE + [dp.tile([128, QB], F32, tag=f"dps{m}",
                                    name=f"dps{m}") for m in range(2, 8)]
            for m in range(2, 8):
                dense_chain(dpss[m], 0, m)
        else:
            dpss = [(dpe if m < 2 else dp).tile(
                [128, QB], F32, tag=f"dps{'E' if m < 2 else ''}{m % 8 if m >= 2 else m}",
                name=f"q1dps{m}") for m in range(8)]
            for m in range(8):
                dense_chain(dpss[m], 1, m)
        for m in range(8):
            evac(dpss[m], qh, m)
        dps = dp.tile([128, QB], F32, tag="dps2", name=f"rag{qh}")
        dense_chain(dps, qh, 8, rows=112)
        evac(dps, qh, 8, rows=112)

    dp.release()
    dpe.release()
    gp1.release()
    op.release()
    gp0.release()
    wdp.release()
    wdpa.release()
    pers.release()
    dram.release()


def build():
    if "nc" in _CACHE:
        return _CACHE["nc"]
    nc = bacc.Bacc("TRN2", target_bir_lowering=False, debug=False,
                   num_devices=NCORES)
    io = {
        "xT": nc.dram_tensor("xT", [D, L], BF16, kind="ExternalInput").ap(),
        "wq3": nc.dram_tensor("wq3", [128, (NMT + 1) * NET * 128], BF16,
                              kind="ExternalInput").ap(),
        "wdT": nc.dram_tensor("wdT", [DPAD, ESH], BF16, kind="ExternalInput").ap(),
        "cosT": nc.dram_tensor("cosT", [128, L], BF16, kind="ExternalInput").ap(),
        "sinT": nc.dram_tensor("sinT", [128, L], BF16, kind="ExternalInput").ap(),
        "rot": nc.dram_tensor("rot", [128, 128], BF16, kind="ExternalInput").ap(),
        "eye": nc.dram_tensor("eye", [128, 128], BF16, kind="ExternalInput").ap(),
        "masks": nc.dram_tensor("masks", [128, 128], BF16,
                                kind="ExternalInput").ap(),
        "ones64": nc.dram_tensor("ones64", [1, 64], BF16,
                                 kind="ExternalInput").ap(),
        "out": nc.dram_tensor("out", [ESH, L], F32, kind="ExternalOutput").ap(),
    }
    with tile.TileContext(nc) as tc:
        _emit(tc, nc, io)
    nc.compile()
    _CACHE["nc"] = nc
    return nc


def make_in_maps(hidden_states, W_qkv, W_dense):
    bf = ml_dtypes.bfloat16
    x = np.asarray(hidden_states, np.float32)
    Wqkv = np.asarray(W_qkv, np.float32)
    Wd = np.asarray(W_dense, np.float32)

    # rope tables, transposed [64, L], replicated to both 64-row halves
    inv = 1.0 / (10000.0 ** (np.arange(0, DKV, 2, dtype=np.float32) / DKV))
    t = np.arange(L, dtype=np.float32)
    freqs = np.outer(t, inv)
    emb = np.concatenate([freqs, freqs], axis=1)          # [L, 64]
    cosT = np.tile(np.cos(emb).T, (2, 1)).astype(bf)      # [128, L]
    sinT = np.tile(np.sin(emb).T, (2, 1)).astype(bf)

    # rotate_half as a matmul: qrot = R1 @ q; lhsT = R1^T; 2-head block diagonal
    R1 = np.zeros((DKV, DKV), np.float32)
    for i in range(32):
        R1[i, i + 32] = -1.0
        R1[i + 32, i] = 1.0
    R2 = np.zeros((128, 128), np.float32)
    R2[:64, :64] = R1
    R2[64:, 64:] = R1
    rot = R2.T.copy().astype(bf)

    eye = np.eye(128, dtype=np.float32).astype(bf)

    ones64 = np.ones((1, 64), np.float32).astype(bf)

    # triangular causal mask for the 128x128 diagonal block
    kk = np.arange(128)[:, None]
    qq = np.arange(128)[None, :]
    masks = (kk <= qq).astype(np.float32).astype(bf)

    # padded dense weights: W_d^T with 64 zero rows appended (pad head),
    # row-tiles permuted to the dense chain order (pair-major over (t, r))
    wdT_full = np.concatenate([Wd.T, np.zeros((DPAD - D, D), np.float32)], axis=0)
    wdT_full = wdT_full.reshape(NET, 128, D)
    perm = [NMT * r + t for (t, r) in CHAIN]
    wdT_full = np.ascontiguousarray(wdT_full[perm]).reshape(DPAD, D).astype(bf)

    in_maps = []
    for c in range(NCORES):
        gg, j = divmod(c, GSZ)
        h0 = HPC * j
        nh = HPC if j < GSZ - 1 else H - HPC * (GSZ - 1)  # 18,18,18,17
        WqT = np.zeros((D, DLOC), np.float32)
        WqT[:, :nh * DKV] = Wqkv[DKV * h0:DKV * (h0 + nh)].T
        # pretile [D, 1280] -> [128, 10*36*128]: slab m holds k-tile i at
        # cols (m*36+i)*128, rows zero-padded to 4608
        Wcat = np.concatenate([WqT, Wqkv[H * DKV:].T], axis=1)   # [D, 1280]
        Wp = np.zeros((NET * 128, (NMT + 1) * 128), np.float32)
        Wp[:D] = Wcat
        wq3 = np.ascontiguousarray(
            Wp.reshape(NET, 128, NMT + 1, 128).transpose(1, 2, 0, 3)
            .reshape(128, (NMT + 1) * NET * 128)).astype(bf)
        in_maps.append({
            "xT": np.ascontiguousarray(x[gg].T).astype(bf),
            "wq3": wq3,
            "wdT": np.ascontiguousarray(wdT_full[:, ESH * j:ESH * (j + 1)]),
            "cosT": cosT, "sinT": sinT, "rot": rot, "eye": eye,
            "masks": masks, "ones64": ones64,
        })
    return in_maps


def assemble(results):
    out = np.empty((N, L, D), np.float32)
    for c in range(NCORES):
        gg, j = divmod(c, GSZ)
        out[gg, :, ESH * j:ESH * (j + 1)] = results[c]["out"].T
    return out


def kernel(hidden_states, W_qkv, W_dense):
    nc = build()
    in_maps = make_in_maps(hidden_states, W_qkv, W_dense)
    res = run_bass_kernel_spmd(nc, in_maps, core_ids=list(range(NCORES)))
    return assemble(res.results)


if __name__ == "__main__":
    import reference
    inputs = reference.setup_inputs()
    out = kernel(**{k: np.asarray(v) for k, v in inputs.items()})
    print("out", out.shape, out.dtype)


# revision 64
# speedup vs baseline: 1.0332x; 1.0002x over previous
"""Distributed MQA attention block (N=2, L=1024, D=4544, H=71, Dkv=64) on 8 TRN2 cores.

Sharding: 2 batch groups x 4-way head tensor-parallel.
  core c = 4*g + j: batch g, heads [18j, 18j+18) (core j=3: 17 real heads + 1 zero pad).

v4 layout (vs v2):
  - ALL attention (both q-halves) is interleaved into the projection pair
    loop, so softmax exp (ScalarE) always hides under the PE-heavy
    projection stream and the PE never starves on exp results.
  - Scores matmuls are K=64 row-packed: head h0 runs on PE row-group (0,0),
    h1 on (64,0) concurrently -- half the scores cycles.
  - AV chains and exp skip causally-dead columns (c0 trim).
  - The AllGather is split into 3 segments (pairs 0-3 / 4-6 / 7-8) fired as
    the pair loop passes them, so every segment lands before dense needs it.
  - ag_in is pair-major ([qh0p0|qh0p1|qh1p0|qh1p1] x 64 rows per pair), and
    the dense contraction runs pair-major (t-major over (t, r)) with
    host-side row-permuted W_dense, so all gather addresses are
    core-independent (SPMD) and early AG segments feed the first chains.
"""

import sys

if "/opt/trn_rl_repo" not in sys.path:
    sys.path.insert(0, "/opt/trn_rl_repo")

import numpy as np
import ml_dtypes

import concourse.bass as bass
import concourse.bacc as bacc
import concourse.mybir as mybir
import concourse.tile as tile
from concourse.bass_utils import run_bass_kernel_spmd

BF16 = mybir.dt.bfloat16
F32 = mybir.dt.float32
AF = mybir.ActivationFunctionType

N, L, D = 2, 1024, 4544
H, DKV = 71, 64
NCORES, GSZ = 8, 4
HPC = 18                 # heads per core (last core of each group: 17 real + 1 pad)
DLOC = HPC * DKV         # 1152
DPAD = GSZ * DLOC        # 4608 = 36 * 128
ESH = D // GSZ           # 1136 output-column shard
NET = 36                 # e-contraction tiles over D=4544 (35 x 128 + 64 zero-pad)
NMT = DLOC // 128        # 9 m-tiles of Q^T rows (2 heads each)
QB = 512                 # q-block (half of L)
SCALE = 1.0 / np.sqrt(DKV)
REPLICA_GROUPS = [[0, 1, 2, 3], [4, 5, 6, 7]]

# AG segments in ag_in row space (pair hp occupies rows [hp*256, hp*256+256)
# as [qh0p0|qh0p1|qh1p0|qh1p1] x 64): one AllGather per pair for pairs 0-7,
# pipelined on the collective stream right behind the pair loop; pair 8 is
# split by q-half and fired as each half's norms land, so the very last
# gathers arrive long before the dense tail reaches them
SEGS = [(p * 256, (p + 1) * 256) for p in range(8)] + [(2176, 2304),
                                                       (2048, 2176)]

# dense chain order: pair-major over (t, r) -> global k-tile 9r+t
CHAIN = [(t, r) for t in range(NMT) for r in range(GSZ)]

_CACHE = {}


def _emit(tc, nc, io):
    xT, wq3, wdT, cosT, sinT, rot, eye, masks, ones64, out = (
        io["xT"], io["wq3"], io["wdT"], io["cosT"], io["sinT"], io["rot"],
        io["eye"], io["masks"], io["ones64"], io["out"],
    )

    # ---- persistent SBUF (live through the whole kernel) ----
    pers = tc.alloc_tile_pool(name="pers", bufs=1)
    qsb = pers.tile([128, NMT * 1024], BF16, tag="qsb")    # roped Q^T, 2 heads/tile
    ksbA = pers.tile([128, 1024], BF16, tag="ksbA")        # [K^T; 0]
    ksbB = pers.tile([128, 1024], BF16, tag="ksbB")        # [0; K^T]
    vsb = pers.tile([128, 8 * 65], BF16, tag="vsb")        # V [tok,64]+ones col
    cossb = pers.tile([128, 1024], BF16, tag="cossb")
    sinsb = pers.tile([128, 1024], BF16, tag="sinsb")
    rotsb = pers.tile([128, 128], BF16, tag="rotsb")
    eyesb = pers.tile([128, 128], BF16, tag="eyesb")
    masksb = pers.tile([128, 128], BF16, tag="masksb")
    onesb = pers.tile([1, 64], BF16, tag="onesb")          # ones lhsT for 1/sum bcast

    # first 12 W_dense k-tiles load during the projection phase (fits SBUF
    # alongside x^T); the rest stream in once x^T frees
    NWA = 12
    wdpa = tc.alloc_tile_pool(name="wdpa", bufs=1)
    wdsbA = wdpa.tile([128, NWA * ESH], BF16, tag="wdsbA")

    # ---- DRAM bounce buffers for the segmented AllGather ----
    # ag_in row block for pair hp: [hp*256 + qh*128 + par*64, +64)
    dram = tc.alloc_tile_pool(name="dram", bufs=1, space="DRAM")
    ag_in = dram.tile([NMT * 256, QB], BF16, tag="agin", name="agin")
    ag_out = [dram.tile([GSZ * (hi - lo), QB], BF16, tag=f"agout{s}",
                        name=f"agout{s}") for s, (lo, hi) in enumerate(SEGS)]

    # =========== Phase AB: projections + interleaved attention ===========
    # PSUM budget (8 banks): qps 2 + scores(sc, 2 bufs) 4 + AV/qrot(ac) 2.
    # Pool releases are LIFO per (space, side).
    xp = tc.alloc_tile_pool(name="xp", bufs=1, side="right")
    wqp = tc.alloc_tile_pool(name="wqp", bufs=3, side="right")
    ra = tc.alloc_tile_pool(name="ra", bufs=2, side="right")
    rt = tc.alloc_tile_pool(name="rt", bufs=1, side="right")
    ex = tc.alloc_tile_pool(name="ex", bufs=14)
    nr = tc.alloc_tile_pool(name="nr", bufs=1)
    # asb staging is double-buffered: the SWDGE ag_in write of one norm chain
    # must not block the next chain's mul from taking the slot
    nra = tc.alloc_tile_pool(name="nra", bufs=2)
    ps = tc.alloc_tile_pool(name="ps", bufs=1, space="PSUM")    # qps slot

    def load_w_cols(dst, m):
        # host-pretiled weights: m-th [128, NET*128] slab is contiguous
        # (k-tile i at cols i*128, tail rows already zero-padded)
        nc.sync.dma_start(dst[:, :], wq3[:, m * NET * 128:(m + 1) * NET * 128])

    sc = tc.alloc_tile_pool(name="sc", bufs=2, space="PSUM", side="right")
    ac = tc.alloc_tile_pool(name="ac", bufs=2, space="PSUM", side="right")

    # ---- startup DMAs: KV + pair-0 weights chunked and interleaved with the
    # first x^T tiles, so the first k-tile matmuls start ~2us in ----
    def load_w_chunk(dst, m, ch, nch):
        step = NET // nch * 128
        nc.sync.dma_start(dst[:, ch * step:(ch + 1) * step],
                          wq3[:, m * NET * 128 + ch * step:
                              m * NET * 128 + (ch + 1) * step])

    wkv = wqp.tile([128, NET * 128], BF16, tag="wm")
    wqm0 = wqp.tile([128, NET * 128], BF16, tag="wm")
    xsb = xp.tile([128, NET * 1024], BF16, tag="xsb")
    nc.vector.memset(xsb[64:128, (NET - 1) * 1024:NET * 1024], 0.0)
    for ch in range(4):
        load_w_chunk(wkv, NMT, ch, 4)
        load_w_chunk(wqm0, 0, ch, 4)
        nc.sync.dma_start(xsb[:, ch * 1024:(ch + 1) * 1024],
                          xT[ch * 128:(ch + 1) * 128, :])
    for i in range(4, NET - 1):
        nc.sync.dma_start(xsb[:, i * 1024:(i + 1) * 1024],
                          xT[i * 128:(i + 1) * 128, :])
        if i == 4:
            nc.sync.dma_start(cossb[:, :], cosT[:, :])
            nc.sync.dma_start(sinsb[:, :], sinT[:, :])
            nc.sync.dma_start(rotsb[:, :], rot[:, :])
            nc.sync.dma_start(eyesb[:, :], eye[:, :])
            nc.sync.dma_start(masksb[:, :], masks[:, :])
            nc.sync.dma_start(onesb[:, :], ones64[:, :])
            nc.vector.memset(vsb[:, :], 1.0)
            nc.vector.memset(ksbA[:, :], 0.0)
            nc.vector.memset(ksbB[:, :], 0.0)
    nc.sync.dma_start(xsb[0:64, (NET - 1) * 1024:NET * 1024],
                      xT[(NET - 1) * 128:D, :])
    for i in range(NWA):
        nc.sync.dma_start(wdsbA[:, i * ESH:(i + 1) * ESH],
                          wdT[i * 128:(i + 1) * 128, :])

    # ---- K/V + pair-0 Q projections interleaved: the first pass over x^T
    # is DMA-arrival-paced, so the pair-0 chain rides along for free.
    # The projection PSUM is split into two per-half tags (qp0/qp1) so a
    # pair's q-half-0 bank frees as soon as its copy lands -- the next
    # pair's pass-0 never waits on this pair's pass-1 copy. ----
    kvps = [ps.tile([128, QB], F32, tag=f"qp{q}", name=f"kv{q}")
            for q in range(2)]
    qps0 = sc.tile([128, 1024], F32, tag="sc", name="qps0")
    for i in range(NET):
        for q in range(2):
            nc.tensor.matmul(
                kvps[q][:, :],
                lhsT=wkv[:, i * 128:(i + 1) * 128],
                rhs=xsb[:, i * 1024 + q * QB:i * 1024 + (q + 1) * QB],
                start=(i == 0), stop=(i == NET - 1),
            )
        for q in range(2):
            nc.tensor.matmul(
                qps0[:, q * QB:(q + 1) * QB],
                lhsT=wqm0[:, i * 128:(i + 1) * 128],
                rhs=xsb[:, i * 1024 + q * QB:i * 1024 + (q + 1) * QB],
                start=(i == 0), stop=(i == NET - 1),
            )
    kvraw = ra.tile([128, 1024], BF16, tag="ra")
    for q in range(2):
        nc.vector.tensor_copy(kvraw[:, q * QB:(q + 1) * QB], kvps[q][:, :])
    qraw0 = ra.tile([128, 1024], BF16, tag="ra")
    nc.vector.tensor_copy(qraw0[:, :], qps0[:, :])
    # rope K (rows 0:64) into ksbA[0:64], then copy into ksbB[64:128]
    krot = [ps.tile([128, QB], F32, tag=f"qp{q}", name=f"kr{q}")
            for q in range(2)]
    for q in range(2):
        nc.tensor.matmul(krot[q][0:64, :],
                         lhsT=rotsb[0:64, 0:64],
                         rhs=kvraw[0:64, q * QB:(q + 1) * QB],
                         start=True, stop=True)
    for q in range(2):
        s = slice(q * QB, (q + 1) * QB)
        t1 = rt.tile([128, QB], F32, tag="t1")
        t2 = rt.tile([128, QB], F32, tag="t2")
        nc.vector.tensor_mul(t1[0:64, :], kvraw[0:64, s], cossb[0:64, s])
        nc.vector.tensor_mul(t2[0:64, :], krot[q][0:64, :], sinsb[0:64, s])
        nc.vector.tensor_add(ksbA[0:64, s], t1[0:64, :], t2[0:64, :])
    nc.sync.dma_start(ksbB[64:128, :], ksbA[0:64, :])

    # V^T (kvraw rows 64:128) -> transpose to V [tok, 64] chunks in vsb,
    # rotating through the (otherwise idle) qp0 slot
    for t8 in range(8):
        vtp = ps.tile([128, 64], BF16, tag="qp0", name=f"vtp{t8}")
        nc.tensor.transpose(vtp[:, :],
                            kvraw[64:128, t8 * 128:(t8 + 1) * 128],
                            eyesb[64:128, 64:128])
        nc.scalar.copy(vsb[:, t8 * 65:t8 * 65 + 64], vtp[:, :])

    def s_tile(qh, hp, kt):
        # scores + exp + mask for one k-tile (both heads). K=64 row-packed:
        # head h0 streams through PE row-group (0,0), h1 through (64,0) --
        # the two matmuls run concurrently on disjoint 64-row halves of the
        # array. Diagonal tile 4*qh+j: columns below 128*j fully masked
        # (never read downstream), the [128j, 128j+128) block gets the
        # triangular mask.
        var = kt - 4 * qh
        c0 = 128 * var if var > 0 else 0
        scp = sc.tile([128, 1024], F32, tag="sc")
        nc.tensor.matmul(
            scp[:, c0:QB],
            lhsT=ksbA[0:64, kt * 128:(kt + 1) * 128],
            rhs=qsb[0:64, hp * 1024 + qh * QB + c0:hp * 1024 + (qh + 1) * QB],
            start=True, stop=True,
        )
        nc.tensor.matmul(
            scp[:, QB + c0:2 * QB],
            lhsT=ksbB[64:128, kt * 128:(kt + 1) * 128],
            rhs=qsb[64:128, hp * 1024 + qh * QB + c0:hp * 1024 + (qh + 1) * QB],
            start=True, stop=True,
        )
        es = ex.tile([128, 1024], BF16, tag="es")
        if c0 == 0:
            nc.scalar.activation(es[:, :], scp[:, :], AF.Exp, scale=SCALE)
        else:
            ev = es.rearrange("p (a b) -> p a b", a=2)
            sv = scp.rearrange("p (a b) -> p a b", a=2)
            nc.scalar.activation(ev[:, :, c0:QB], sv[:, :, c0:QB],
                                 AF.Exp, scale=SCALE)
        if var >= 0:
            for par in range(2):
                nc.vector.tensor_mul(
                    es[:, par * QB + c0:par * QB + c0 + 128],
                    es[:, par * QB + c0:par * QB + c0 + 128],
                    masksb[:, 0:128])
        return es

    def av_chain(qh, par, ess, acc, kts):
        # causal trim: es tile kt is zero (masked) for local queries < c0,
        # so the accumulating matmul skips those columns (kt=0 has c0=0 and
        # start=True initializes the full bank range).
        nkt = 4 * qh + 4
        for kt in kts:
            var = kt - 4 * qh
            c0 = 128 * var if var > 0 else 0
            nc.tensor.matmul(
                acc[0:65, c0:QB],
                lhsT=vsb[:, kt * 65:(kt + 1) * 65],
                rhs=ess[kt][:, par * QB + c0:(par + 1) * QB],
                start=(kt == 0), stop=(kt == nkt - 1),
            )

    def norm_pre(par, acc):
        # softmax 1/sum on VectorE; priority-boosted -- these three tiny ops
        # gate the rbp broadcast matmul, which otherwise stalls the PE in the
        # late pairs when the projection filler is exhausted
        with tc.high_priority():
            stg = nr.tile([1, QB], F32, tag=f"stg{par}", name=f"stg{par}")
            nc.vector.tensor_copy(stg[:, :], acc[64:65, :])
            rcf = nr.tile([1, QB], F32, tag=f"rcf{par}", name=f"rcf{par}")
            nc.vector.reciprocal_approx_fast(rcf[:, :], stg[:, :])
            rc = nr.tile([1, QB], BF16, tag=f"rc{par}", name=f"rc{par}")
            with nc.allow_low_precision(reason="softmax 1/sum in bf16"):
                nc.vector.tensor_copy(rc[:, :], rcf[:, :])
        return rc

    def norm_post(qh, hp, par, acc, rc):
        rbp = sc.tile([128, 1024], F32, tag="sc")
        nc.tensor.matmul(rbp[0:64, 0:QB], lhsT=onesb[0:1, :],
                         rhs=rc[0:1, :], start=True, stop=True)
        rbs = nr.tile([64, QB], BF16, tag=f"rbs{par}", name=f"rbs{par}")
        nc.vector.tensor_copy(rbs[:, :], rbp[0:64, 0:QB])
        asb = nra.tile([64, QB], BF16, tag=f"asb{par}", name=f"asb{par}")
        nc.vector.tensor_mul(asb[:, :], acc[0:64, :], rbs[:, :])
        # SWDGE (gpsimd) DMA: completion semaphores separate from the shared
        # HWDGE queues, so the AllGather trigger thresholds only count these
        row = hp * 256 + qh * 128 + par * 64
        nc.gpsimd.dma_start(ag_in[row:row + 64, :], asb[:, :])

    def attn(qh, hp):
        nkt = 4 * qh + 4
        ess = [s_tile(qh, hp, kt) for kt in range(nkt)]
        accs, rcs = [], []
        for par in range(2):
            acc = ac.tile([128, QB], F32, tag="ac")
            av_chain(qh, par, ess, acc, range(nkt))
            rcs.append(norm_pre(par, acc))
            accs.append(acc)
        for par in range(2):
            norm_post(qh, hp, par, accs[par], rcs[par])

    def rope_half(hp, q, qraw, qr):
        # the three DVE ops are priority-boosted: the final qsb add gates all
        # 24 scores matmuls of the pair, and must not queue behind bulk
        # mask-mul/norm DVE work in the late pairs
        s = slice(q * QB, (q + 1) * QB)
        nc.tensor.matmul(qr[:, :], lhsT=rotsb[:, :], rhs=qraw[:, s],
                         start=True, stop=True)
        with tc.high_priority():
            t1 = rt.tile([128, QB], F32, tag="t1")
            t2 = rt.tile([128, QB], F32, tag="t2")
            nc.vector.tensor_mul(t1[:, :], qraw[:, s], cossb[:, s])
            nc.vector.tensor_mul(t2[:, :], qr[:, :], sinsb[:, s])
            nc.vector.tensor_add(
                qsb[:, hp * 1024 + q * QB:hp * 1024 + (q + 1) * QB],
                t1[:, :], t2[:, :])

    def fire_ag(seg):
        lo, hi = SEGS[seg]
        nc.gpsimd.collective_compute(
            "AllGather", mybir.AluOpType.bypass,
            ins=[ag_in[lo:hi, :].opt()],
            outs=[ag_out[seg].opt()],
            replica_groups=REPLICA_GROUPS,
        )

    # ---- pair-0 RoPE + attention (its projection rode the KV window) ----
    qrots0 = [ac.tile([128, QB], F32, tag="ac", name=f"qrot0{q}")
              for q in range(2)]
    rope_half(0, 0, qraw0, qrots0[0])
    rope_half(0, 1, qraw0, qrots0[1])
    attn(1, 0)
    attn(0, 0)
    fire_ag(0)

    # ---- Q projection + RoPE + full attention, per m-tile pair ----
    for hp in range(1, NMT):
        wqm = wqp.tile([128, NET * 128], BF16, tag="wm")
        load_w_cols(wqm, hp)
        qps = [ps.tile([128, QB], F32, tag=f"qp{q}", name=f"qps{q}")
               for q in range(2)]
        qraw = ra.tile([128, 1024], BF16, tag="ra")
        qrots = [ac.tile([128, QB], F32, tag="ac", name=f"qrot{q}")
                 for q in range(2)]
        for i in range(NET):
            nc.tensor.matmul(
                qps[0][:, :], lhsT=wqm[:, i * 128:(i + 1) * 128],
                rhs=xsb[:, i * 1024:i * 1024 + QB],
                start=(i == 0), stop=(i == NET - 1),
            )
        with tc.high_priority():
            nc.vector.tensor_copy(qraw[:, 0:QB], qps[0][:, :])
        for i in range(NET):
            nc.tensor.matmul(
                qps[1][:, :], lhsT=wqm[:, i * 128:(i + 1) * 128],
                rhs=xsb[:, i * 1024 + QB:i * 1024 + 2 * QB],
                start=(i == 0), stop=(i == NET - 1),
            )
            if i == 5:
                rope_half(hp, 0, qraw, qrots[0])
        # only the PSUM->SBUF copies are priority-boosted (they gate the rope
        # -> scores chain and must not queue behind bulk DVE/ACT work); PE
        # attention ops keep natural order so the static schedule interleaves
        # them with the next pair's projection stream
        with tc.high_priority():
            nc.vector.tensor_copy(qraw[:, QB:2 * QB], qps[1][:, :])
        rope_half(hp, 1, qraw, qrots[1])
        attn(1, hp)
        if hp == NMT - 1:
            fire_ag(8)      # pair-8 q-half-1 rows, as soon as they land
        attn(0, hp)
        fire_ag(9 if hp == NMT - 1 else hp)
        if hp == NMT - 2:
            # scheduler fence BEFORE the last pair: dense work may interleave
            # with pair-8's attention tail, but cannot be hoisted ahead of the
            # earlier projection stream / AG triggers
            tc.no_sync_barrier()

    # free x^T/W_q SBUF; W_dense loads stream in under the attention tail
    rt.release()
    ra.release()
    wqp.release()
    xp.release()
    wdp = tc.alloc_tile_pool(name="wdp", bufs=1, side="right")
    wdsbB = wdp.tile([128, (NET - NWA) * ESH], BF16, tag="wdsbB")
    for i in range(NET - NWA):
        # scalar (ACT) HWDGE ring: keeps these 7MB of loads off the sync
        # ring so the dense gather DMAs are not queued behind them
        nc.scalar.dma_start(wdsbB[:, i * ESH:(i + 1) * ESH],
                            wdT[(NWA + i) * 128:(NWA + i + 1) * 128, :])

    # =========== dense: out^T[e_shard, q] = W_d^T[dpad, e].T @ attn^T ===========
    # wdT rows are host-permuted to chain order (pair-major over (t, r)),
    # so chain position p contracts global k-tile 9r+t.
    # The projection PSUM (ps) frees as soon as pair-8's projection is copied
    # out, so two dense chains (dpe) can run in the ACT-bound attention tail;
    # the remaining six (dp) start once the attention PSUM pools release.
    ps.release()
    dpe = tc.alloc_tile_pool(name="dpe", bufs=1, space="PSUM")
    gp0 = tc.alloc_tile_pool(name="gp0", bufs=1, side="right")
    op = tc.alloc_tile_pool(name="op", bufs=2, side="right")

    def gather_src(qh, t, r):
        row = t * 256 + qh * 128
        seg = next(s for s, (lo, hi) in enumerate(SEGS) if lo <= row < hi)
        lo, hi = SEGS[seg]
        srow = r * (hi - lo) + (row - lo)
        return ag_out[seg][srow:srow + 128, :]

    # qh0's gather buffer fits alongside the still-live es pool; qh1's is
    # allocated after the attention pools release
    gath = [gp0.tile([128, NET * QB], BF16, tag="gath0", name="gath0"), None]

    def emit_gathers(qh):
        # per-tile gather DMAs in chain order: chain MMs wait only on their
        # own tile, so the i-minor chains start on the first landed tile
        for p, (t, r) in enumerate(CHAIN):
            nc.sync.dma_start(gath[qh][:, p * QB:(p + 1) * QB],
                              gather_src(qh, t, r))

    emit_gathers(0)

    def dense_chain(dtile, qh, m, rows=128):
        for i in range(NET):
            wds, ii = (wdsbA, i) if i < NWA else (wdsbB, i - NWA)
            nc.tensor.matmul(
                dtile[0:rows, :],
                lhsT=wds[:, ii * ESH + m * 128:ii * ESH + m * 128 + rows],
                rhs=gath[qh][:, i * QB:(i + 1) * QB],
                start=(i == 0), stop=(i == NET - 1),
            )

    def evac(dtile, qh, m, rows=128):
        osb = op.tile([128, QB], F32, tag="op")
        nc.scalar.copy(osb[0:rows, :], dtile[0:rows, :])
        nc.sync.dma_start(out[m * 128:m * 128 + rows, qh * QB:(qh + 1) * QB],
                          osb[0:rows, :])

    # early chains m=0,1 of q-half 0 -- fill the attention-tail PE bubbles
    dpssE = [dpe.tile([128, QB], F32, tag=f"dpsE{m}", name=f"dpsE{m}")
             for m in range(2)]
    for m in range(2):
        dense_chain(dpssE[m], 0, m)

    ac.release()
    sc.release()
    nra.release()
    nr.release()
    ex.release()
    dp = tc.alloc_tile_pool(name="dp", bufs=1, space="PSUM")
    gp1 = tc.alloc_tile_pool(name="gp1", bufs=1, side="right")
    gath[1] = gp1.tile([128, NET * QB], BF16, tag="gath1", name="gath1")
    emit_gathers(1)

    # i-minor dense: parallel accumulation chains (one PSUM bank each) so the
    # first gather tiles feed all chains and the DMA stays ahead of the PE
    for qh in range(2):
        if qh == 0:
            dpss = dpssE + [dp.tile([128, QB], F32, tag=f"dps{m}",
                                    name=f"dps{m}") for m in range(2, 8)]
            for m in range(2, 8):
                dense_chain(dpss[m], 0, m)
        else:
            dpss = [(dpe if m < 2 else dp).tile(
                [128, QB], F32, tag=f"dps{'E' if m < 2 else ''}{m % 8 if m >= 2 else m}",
                name=f"q1dps{m}") for m in range(8)]
            for m in range(8):
                dense_chain(dpss[m], 1, m)
        for m in range(8):
            evac(dpss[m], qh, m)
        dps = dp.tile([128, QB], F32, tag="dps2", name=f"rag{qh}")
        dense_chain(dps, qh, 8, rows=112)
        evac(dps, qh, 8, rows=112)

    dp.release()
    dpe.release()
    gp1.release()
    op.release()
    gp0.release()
    wdp.release()
    wdpa.release()
    pers.release()
    dram.release()


def build():
    if "nc" in _CACHE:
        return _CACHE["nc"]
    nc = bacc.Bacc("TRN2", target_bir_lowering=False, debug=False,
                   num_devices=NCORES)
    io = {
        "xT": nc.dram_tensor("xT", [D, L], BF16, kind="ExternalInput").ap(),
        "wq3": nc.dram_tensor("wq3", [128, (NMT + 1) * NET * 128], BF16,
                              kind="ExternalInput").ap(),
        "wdT": nc.dram_tensor("wdT", [DPAD, ESH], BF16, kind="ExternalInput").ap(),
        "cosT": nc.dram_tensor("cosT", [128, L], BF16, kind="ExternalInput").ap(),
        "sinT": nc.dram_tensor("sinT", [128, L], BF16, kind="ExternalInput").ap(),
        "rot": nc.dram_tensor("rot", [128, 128], BF16, kind="ExternalInput").ap(),
        "eye": nc.dram_tensor("eye", [128, 128], BF16, kind="ExternalInput").ap(),
        "masks": nc.dram_tensor("masks", [128, 128], BF16,
                                kind="ExternalInput").ap(),
        "ones64": nc.dram_tensor("ones64", [1, 64], BF16,
                                 kind="ExternalInput").ap(),
        "out": nc.dram_tensor("out", [ESH, L], F32, kind="ExternalOutput").ap(),
    }
    with tile.TileContext(nc) as tc:
        _emit(tc, nc, io)
    nc.compile()
    _CACHE["nc"] = nc
    return nc


def make_in_maps(hidden_states, W_qkv, W_dense):
    bf = ml_dtypes.bfloat16
    x = np.asarray(hidden_states, np.float32)
    Wqkv = np.asarray(W_qkv, np.float32)
    Wd = np.asarray(W_dense, np.float32)

    # rope tables, transposed [64, L], replicated to both 64-row halves
    inv = 1.0 / (10000.0 ** (np.arange(0, DKV, 2, dtype=np.float32) / DKV))
    t = np.arange(L, dtype=np.float32)
    freqs = np.outer(t, inv)
    emb = np.concatenate([freqs, freqs], axis=1)          # [L, 64]
    cosT = np.tile(np.cos(emb).T, (2, 1)).astype(bf)      # [128, L]
    sinT = np.tile(np.sin(emb).T, (2, 1)).astype(bf)

    # rotate_half as a matmul: qrot = R1 @ q; lhsT = R1^T; 2-head block diagonal
    R1 = np.zeros((DKV, DKV), np.float32)
    for i in range(32):
        R1[i, i + 32] = -1.0
        R1[i + 32, i] = 1.0
    R2 = np.zeros((128, 128), np.float32)
    R2[:64, :64] = R1
    R2[64:, 64:] = R1
    rot = R2.T.copy().astype(bf)

    eye = np.eye(128, dtype=np.float32).astype(bf)

    ones64 = np.ones((1, 64), np.float32).astype(bf)

    # triangular causal mask for the 128x128 diagonal block
    kk = np.arange(128)[:, None]
    qq = np.arange(128)[None, :]
    masks = (kk <= qq).astype(np.float32).astype(bf)

    # padded dense weights: W_d^T with 64 zero rows appended (pad head),
    # row-tiles permuted to the dense chain order (pair-major over (t, r))
    wdT_full = np.concatenate([Wd.T, np.zeros((DPAD - D, D), np.float32)], axis=0)
    wdT_full = wdT_full.reshape(NET, 128, D)
    perm = [NMT * r + t for (t, r) in CHAIN]
    wdT_full = np.ascontiguousarray(wdT_full[perm]).reshape(DPAD, D).astype(bf)

    in_maps = []
    for c in range(NCORES):
        gg, j = divmod(c, GSZ)
        h0 = HPC * j
        nh = HPC if j < GSZ - 1 else H - HPC * (GSZ - 1)  # 18,18,18,17
        WqT = np.zeros((D, DLOC), np.float32)
        WqT[:, :nh * DKV] = Wqkv[DKV * h0:DKV * (h0 + nh)].T
        # pretile [D, 1280] -> [128, 10*36*128]: slab m holds k-tile i at
        # cols (m*36+i)*128, rows zero-padded to 4608
        Wcat = np.concatenate([WqT, Wqkv[H * DKV:].T], axis=1)   # [D, 1280]
        Wp = np.zeros((NET * 128, (NMT + 1) * 128), np.float32)
        Wp[:D] = Wcat
        wq3 = np.ascontiguousarray(
            Wp.reshape(NET, 128, NMT + 1, 128).transpose(1, 2, 0, 3)
            .reshape(128, (NMT + 1) * NET * 128)).astype(bf)
        in_maps.append({
            "xT": np.ascontiguousarray(x[gg].T).astype(bf),
            "wq3": wq3,
            "wdT": np.ascontiguousarray(wdT_full[:, ESH * j:ESH * (j + 1)]),
            "cosT": cosT, "sinT": sinT, "rot": rot, "eye": eye,
            "masks": masks, "ones64": ones64,
        })
    return in_maps


def assemble(results):
    out = np.empty((N, L, D), np.float32)
    for c in range(NCORES):
        gg, j = divmod(c, GSZ)
        out[gg, :, ESH * j:ESH * (j + 1)] = results[c]["out"].T
    return out


def kernel(hidden_states, W_qkv, W_dense):
    nc = build()
    in_maps = make_in_maps(hidden_states, W_qkv, W_dense)
    res = run_bass_kernel_spmd(nc, in_maps, core_ids=list(range(NCORES)))
    return assemble(res.results)


if __name__ == "__main__":
    import reference
    inputs = reference.setup_inputs()
    out = kernel(**{k: np.asarray(v) for k, v in inputs.items()})
    print("out", out.shape, out.dtype)


# revision 69
# speedup vs baseline: 1.0359x; 1.0025x over previous
"""Distributed MQA attention block (N=2, L=1024, D=4544, H=71, Dkv=64) on 8 TRN2 cores.

Sharding: 2 batch groups x 4-way head tensor-parallel.
  core c = 4*g + j: batch g, heads [18j, 18j+18) (core j=3: 17 real heads + 1 zero pad).

v4 layout (vs v2):
  - ALL attention (both q-halves) is interleaved into the projection pair
    loop, so softmax exp (ScalarE) always hides under the PE-heavy
    projection stream and the PE never starves on exp results.
  - Scores matmuls are K=64 row-packed: head h0 runs on PE row-group (0,0),
    h1 on (64,0) concurrently -- half the scores cycles.
  - AV chains and exp skip causally-dead columns (c0 trim).
  - The AllGather is split into 3 segments (pairs 0-3 / 4-6 / 7-8) fired as
    the pair loop passes them, so every segment lands before dense needs it.
  - ag_in is pair-major ([qh0p0|qh0p1|qh1p0|qh1p1] x 64 rows per pair), and
    the dense contraction runs pair-major (t-major over (t, r)) with
    host-side row-permuted W_dense, so all gather addresses are
    core-independent (SPMD) and early AG segments feed the first chains.
"""

import sys

if "/opt/trn_rl_repo" not in sys.path:
    sys.path.insert(0, "/opt/trn_rl_repo")

import numpy as np
import ml_dtypes

import concourse.bass as bass
import concourse.bacc as bacc
import concourse.mybir as mybir
import concourse.tile as tile
from concourse.bass_utils import run_bass_kernel_spmd

BF16 = mybir.dt.bfloat16
F32 = mybir.dt.float32
AF = mybir.ActivationFunctionType

N, L, D = 2, 1024, 4544
H, DKV = 71, 64
NCORES, GSZ = 8, 4
HPC = 18                 # heads per core (last core of each group: 17 real + 1 pad)
DLOC = HPC * DKV         # 1152
DPAD = GSZ * DLOC        # 4608 = 36 * 128
ESH = D // GSZ           # 1136 output-column shard
NET = 36                 # e-contraction tiles over D=4544 (35 x 128 + 64 zero-pad)
NMT = DLOC // 128        # 9 m-tiles of Q^T rows (2 heads each)
QB = 512                 # q-block (half of L)
SCALE = 1.0 / np.sqrt(DKV)
REPLICA_GROUPS = [[0, 1, 2, 3], [4, 5, 6, 7]]

# AG segments in ag_in row space (pair hp occupies rows [hp*256, hp*256+256)
# as [qh0p0|qh0p1|qh1p0|qh1p1] x 64): one AllGather per pair for pairs 0-7,
# pipelined on the collective stream right behind the pair loop; pair 8 is
# split by q-half and fired as each half's norms land, so the very last
# gathers arrive long before the dense tail reaches them
SEGS = [(p * 256, (p + 1) * 256) for p in range(8)] + [(2176, 2304),
                                                       (2048, 2176)]

# dense chain order: pair-major over (t, r) -> global k-tile 9r+t
CHAIN = [(t, r) for t in range(NMT) for r in range(GSZ)]

_CACHE = {}


def _emit(tc, nc, io):
    xT, wq3, wdT, cosT, sinT, rot, eye, masks, ones64, out = (
        io["xT"], io["wq3"], io["wdT"], io["cosT"], io["sinT"], io["rot"],
        io["eye"], io["masks"], io["ones64"], io["out"],
    )

    # ---- persistent SBUF (live through the whole kernel) ----
    pers = tc.alloc_tile_pool(name="pers", bufs=1)
    qsb = pers.tile([128, NMT * 1024], BF16, tag="qsb")    # roped Q^T, 2 heads/tile
    ksbA = pers.tile([128, 1024], BF16, tag="ksbA")        # [K^T; 0]
    ksbB = pers.tile([128, 1024], BF16, tag="ksbB")        # [0; K^T]
    vsb = pers.tile([128, 8 * 65], BF16, tag="vsb")        # V [tok,64]+ones col
    cossb = pers.tile([128, 1024], BF16, tag="cossb")
    sinsb = pers.tile([128, 1024], BF16, tag="sinsb")
    rotsb = pers.tile([128, 128], BF16, tag="rotsb")
    eyesb = pers.tile([128, 128], BF16, tag="eyesb")
    masksb = pers.tile([128, 128], BF16, tag="masksb")
    onesb = pers.tile([1, 64], BF16, tag="onesb")          # ones lhsT for 1/sum bcast

    # first 12 W_dense k-tiles load during the projection phase (fits SBUF
    # alongside x^T); the rest stream in once x^T frees
    NWA = 12
    wdpa = tc.alloc_tile_pool(name="wdpa", bufs=1)
    wdsbA = wdpa.tile([128, NWA * ESH], BF16, tag="wdsbA")

    # ---- DRAM bounce buffers for the segmented AllGather ----
    # ag_in row block for pair hp: [hp*256 + qh*128 + par*64, +64)
    dram = tc.alloc_tile_pool(name="dram", bufs=1, space="DRAM")
    ag_in = dram.tile([NMT * 256, QB], BF16, tag="agin", name="agin")
    ag_out = [dram.tile([GSZ * (hi - lo), QB], BF16, tag=f"agout{s}",
                        name=f"agout{s}") for s, (lo, hi) in enumerate(SEGS)]

    # =========== Phase AB: projections + interleaved attention ===========
    # PSUM budget (8 banks): qps 2 + scores(sc, 2 bufs) 4 + AV/qrot(ac) 2.
    # Pool releases are LIFO per (space, side).
    xp = tc.alloc_tile_pool(name="xp", bufs=1, side="right")
    wqp = tc.alloc_tile_pool(name="wqp", bufs=3, side="right")
    ra = tc.alloc_tile_pool(name="ra", bufs=2, side="right")
    rt = tc.alloc_tile_pool(name="rt", bufs=1, side="right")
    ex = tc.alloc_tile_pool(name="ex", bufs=14)
    nr = tc.alloc_tile_pool(name="nr", bufs=1)
    # asb staging is double-buffered: the SWDGE ag_in write of one norm chain
    # must not block the next chain's mul from taking the slot
    nra = tc.alloc_tile_pool(name="nra", bufs=2)
    ps = tc.alloc_tile_pool(name="ps", bufs=1, space="PSUM")    # qps slot

    def load_w_cols(dst, m):
        # host-pretiled weights: m-th [128, NET*128] slab is contiguous
        # (k-tile i at cols i*128, tail rows already zero-padded)
        nc.sync.dma_start(dst[:, :], wq3[:, m * NET * 128:(m + 1) * NET * 128])

    sc = tc.alloc_tile_pool(name="sc", bufs=2, space="PSUM", side="right")
    ac = tc.alloc_tile_pool(name="ac", bufs=2, space="PSUM", side="right")

    # ---- startup DMAs: KV + pair-0 weights chunked and interleaved with the
    # first x^T tiles, so the first k-tile matmuls start ~2us in ----
    def load_w_chunk(dst, m, ch, nch):
        step = NET // nch * 128
        nc.sync.dma_start(dst[:, ch * step:(ch + 1) * step],
                          wq3[:, m * NET * 128 + ch * step:
                              m * NET * 128 + (ch + 1) * step])

    wkv = wqp.tile([128, NET * 128], BF16, tag="wm")
    wqm0 = wqp.tile([128, NET * 128], BF16, tag="wm")
    xsb = xp.tile([128, NET * 1024], BF16, tag="xsb")
    nc.vector.memset(xsb[64:128, (NET - 1) * 1024:NET * 1024], 0.0)
    for ch in range(4):
        load_w_chunk(wkv, NMT, ch, 4)
        load_w_chunk(wqm0, 0, ch, 4)
        nc.sync.dma_start(xsb[:, ch * 1024:(ch + 1) * 1024],
                          xT[ch * 128:(ch + 1) * 128, :])
    for i in range(4, NET - 1):
        nc.sync.dma_start(xsb[:, i * 1024:(i + 1) * 1024],
                          xT[i * 128:(i + 1) * 128, :])
        if i == 4:
            nc.sync.dma_start(cossb[:, :], cosT[:, :])
            nc.sync.dma_start(sinsb[:, :], sinT[:, :])
            nc.sync.dma_start(rotsb[:, :], rot[:, :])
            nc.sync.dma_start(eyesb[:, :], eye[:, :])
            nc.sync.dma_start(masksb[:, :], masks[:, :])
            nc.sync.dma_start(onesb[:, :], ones64[:, :])
            nc.vector.memset(vsb[:, :], 1.0)
            nc.vector.memset(ksbA[:, :], 0.0)
            nc.vector.memset(ksbB[:, :], 0.0)
    nc.sync.dma_start(xsb[0:64, (NET - 1) * 1024:NET * 1024],
                      xT[(NET - 1) * 128:D, :])
    for i in range(NWA):
        nc.sync.dma_start(wdsbA[:, i * ESH:(i + 1) * ESH],
                          wdT[i * 128:(i + 1) * 128, :])

    # ---- K/V + pair-0 Q projections interleaved: the first pass over x^T
    # is DMA-arrival-paced, so the pair-0 chain rides along for free.
    # The projection PSUM is split into two per-half tags (qp0/qp1) so a
    # pair's q-half-0 bank frees as soon as its copy lands -- the next
    # pair's pass-0 never waits on this pair's pass-1 copy. ----
    kvps = [ps.tile([128, QB], F32, tag=f"qp{q}", name=f"kv{q}")
            for q in range(2)]
    qps0 = sc.tile([128, 1024], F32, tag="sc", name="qps0")
    for i in range(NET):
        for q in range(2):
            nc.tensor.matmul(
                kvps[q][:, :],
                lhsT=wkv[:, i * 128:(i + 1) * 128],
                rhs=xsb[:, i * 1024 + q * QB:i * 1024 + (q + 1) * QB],
                start=(i == 0), stop=(i == NET - 1),
            )
        for q in range(2):
            nc.tensor.matmul(
                qps0[:, q * QB:(q + 1) * QB],
                lhsT=wqm0[:, i * 128:(i + 1) * 128],
                rhs=xsb[:, i * 1024 + q * QB:i * 1024 + (q + 1) * QB],
                start=(i == 0), stop=(i == NET - 1),
            )
    kvraw = ra.tile([128, 1024], BF16, tag="ra")
    for q in range(2):
        nc.vector.tensor_copy(kvraw[:, q * QB:(q + 1) * QB], kvps[q][:, :])
    qraw0 = ra.tile([128, 1024], BF16, tag="ra")
    nc.vector.tensor_copy(qraw0[:, :], qps0[:, :])
    # rope K (rows 0:64) into ksbA[0:64], then copy into ksbB[64:128]
    krot = [ps.tile([128, QB], F32, tag=f"qp{q}", name=f"kr{q}")
            for q in range(2)]
    for q in range(2):
        nc.tensor.matmul(krot[q][0:64, :],
                         lhsT=rotsb[0:64, 0:64],
                         rhs=kvraw[0:64, q * QB:(q + 1) * QB],
                         start=True, stop=True)
    for q in range(2):
        s = slice(q * QB, (q + 1) * QB)
        t1 = rt.tile([128, QB], F32, tag="t1")
        t2 = rt.tile([128, QB], F32, tag="t2")
        nc.vector.tensor_mul(t1[0:64, :], kvraw[0:64, s], cossb[0:64, s])
        nc.vector.tensor_mul(t2[0:64, :], krot[q][0:64, :], sinsb[0:64, s])
        nc.vector.tensor_add(ksbA[0:64, s], t1[0:64, :], t2[0:64, :])
    nc.sync.dma_start(ksbB[64:128, :], ksbA[0:64, :])

    # V^T (kvraw rows 64:128) -> transpose to V [tok, 64] chunks in vsb,
    # rotating through the (otherwise idle) qp0 slot
    for t8 in range(8):
        vtp = ps.tile([128, 64], BF16, tag="qp0", name=f"vtp{t8}")
        nc.tensor.transpose(vtp[:, :],
                            kvraw[64:128, t8 * 128:(t8 + 1) * 128],
                            eyesb[64:128, 64:128])
        nc.scalar.copy(vsb[:, t8 * 65:t8 * 65 + 64], vtp[:, :])

    def s_tile(qh, hp, kt):
        # scores + exp + mask for one k-tile (both heads). K=64 row-packed:
        # head h0 streams through PE row-group (0,0), h1 through (64,0) --
        # the two matmuls run concurrently on disjoint 64-row halves of the
        # array. Diagonal tile 4*qh+j: columns below 128*j fully masked
        # (never read downstream), the [128j, 128j+128) block gets the
        # triangular mask.
        var = kt - 4 * qh
        c0 = 128 * var if var > 0 else 0
        scp = sc.tile([128, 1024], F32, tag="sc")
        nc.tensor.matmul(
            scp[:, c0:QB],
            lhsT=ksbA[0:64, kt * 128:(kt + 1) * 128],
            rhs=qsb[0:64, hp * 1024 + qh * QB + c0:hp * 1024 + (qh + 1) * QB],
            start=True, stop=True,
        )
        nc.tensor.matmul(
            scp[:, QB + c0:2 * QB],
            lhsT=ksbB[64:128, kt * 128:(kt + 1) * 128],
            rhs=qsb[64:128, hp * 1024 + qh * QB + c0:hp * 1024 + (qh + 1) * QB],
            start=True, stop=True,
        )
        es = ex.tile([128, 1024], BF16, tag="es")
        if c0 == 0:
            nc.scalar.activation(es[:, :], scp[:, :], AF.Exp, scale=SCALE)
        else:
            ev = es.rearrange("p (a b) -> p a b", a=2)
            sv = scp.rearrange("p (a b) -> p a b", a=2)
            nc.scalar.activation(ev[:, :, c0:QB], sv[:, :, c0:QB],
                                 AF.Exp, scale=SCALE)
        if var >= 0:
            for par in range(2):
                nc.vector.tensor_mul(
                    es[:, par * QB + c0:par * QB + c0 + 128],
                    es[:, par * QB + c0:par * QB + c0 + 128],
                    masksb[:, 0:128])
        return es

    def av_chain(qh, par, ess, acc, kts):
        # causal trim: es tile kt is zero (masked) for local queries < c0,
        # so the accumulating matmul skips those columns (kt=0 has c0=0 and
        # start=True initializes the full bank range).
        nkt = 4 * qh + 4
        for kt in kts:
            var = kt - 4 * qh
            c0 = 128 * var if var > 0 else 0
            nc.tensor.matmul(
                acc[0:65, c0:QB],
                lhsT=vsb[:, kt * 65:(kt + 1) * 65],
                rhs=ess[kt][:, par * QB + c0:(par + 1) * QB],
                start=(kt == 0), stop=(kt == nkt - 1),
            )

    def norm_pre(par, acc):
        # softmax 1/sum on VectorE; priority-boosted -- these three tiny ops
        # gate the rbp broadcast matmul, which otherwise stalls the PE in the
        # late pairs when the projection filler is exhausted
        with tc.high_priority():
            stg = nr.tile([1, QB], F32, tag=f"stg{par}", name=f"stg{par}")
            nc.vector.tensor_copy(stg[:, :], acc[64:65, :])
            rcf = nr.tile([1, QB], F32, tag=f"rcf{par}", name=f"rcf{par}")
            nc.vector.reciprocal_approx_fast(rcf[:, :], stg[:, :])
            rc = nr.tile([1, QB], BF16, tag=f"rc{par}", name=f"rc{par}")
            with nc.allow_low_precision(reason="softmax 1/sum in bf16"):
                nc.vector.tensor_copy(rc[:, :], rcf[:, :])
        return rc

    def norm_post(qh, hp, par, acc, rc):
        rbp = sc.tile([128, 1024], F32, tag="sc")
        nc.tensor.matmul(rbp[0:64, 0:QB], lhsT=onesb[0:1, :],
                         rhs=rc[0:1, :], start=True, stop=True)
        rbs = nr.tile([64, QB], BF16, tag=f"rbs{par}", name=f"rbs{par}")
        nc.vector.tensor_copy(rbs[:, :], rbp[0:64, 0:QB])
        asb = nra.tile([64, QB], BF16, tag=f"asb{par}", name=f"asb{par}")
        nc.vector.tensor_mul(asb[:, :], acc[0:64, :], rbs[:, :])
        # SWDGE (gpsimd) DMA: completion semaphores separate from the shared
        # HWDGE queues, so the AllGather trigger thresholds only count these
        row = hp * 256 + qh * 128 + par * 64
        nc.gpsimd.dma_start(ag_in[row:row + 64, :], asb[:, :])

    def attn(qh, hp, deferred_norms=None):
        # qh1 returns its norm_posts for deferral into qh0's scores phase:
        # the 1/sum DVE chains get ~2us more slack before the rbp matmuls
        # need them, instead of stalling the PE in the late pairs
        nkt = 4 * qh + 4
        ess = []
        for kt in range(nkt):
            ess.append(s_tile(qh, hp, kt))
            if kt == 1 and deferred_norms:
                for t in deferred_norms:
                    norm_post(*t)
                deferred_norms = None
        accs, rcs = [], []
        for par in range(2):
            acc = ac.tile([128, QB], F32, tag="ac")
            av_chain(qh, par, ess, acc, range(nkt))
            rcs.append(norm_pre(par, acc))
            accs.append(acc)
        if qh == 1:
            return [(1, hp, par, accs[par], rcs[par]) for par in range(2)]
        for par in range(2):
            norm_post(qh, hp, par, accs[par], rcs[par])
        return None

    def rope_half(hp, q, qraw, qr):
        # the three DVE ops are priority-boosted: the final qsb add gates all
        # 24 scores matmuls of the pair, and must not queue behind bulk
        # mask-mul/norm DVE work in the late pairs
        s = slice(q * QB, (q + 1) * QB)
        nc.tensor.matmul(qr[:, :], lhsT=rotsb[:, :], rhs=qraw[:, s],
                         start=True, stop=True)
        with tc.high_priority():
            t1 = rt.tile([128, QB], F32, tag="t1")
            t2 = rt.tile([128, QB], F32, tag="t2")
            nc.vector.tensor_mul(t1[:, :], qraw[:, s], cossb[:, s])
            nc.vector.tensor_mul(t2[:, :], qr[:, :], sinsb[:, s])
            nc.vector.tensor_add(
                qsb[:, hp * 1024 + q * QB:hp * 1024 + (q + 1) * QB],
                t1[:, :], t2[:, :])

    def fire_ag(seg):
        lo, hi = SEGS[seg]
        nc.gpsimd.collective_compute(
            "AllGather", mybir.AluOpType.bypass,
            ins=[ag_in[lo:hi, :].opt()],
            outs=[ag_out[seg].opt()],
            replica_groups=REPLICA_GROUPS,
        )

    # ---- pair-0 RoPE + attention (its projection rode the KV window) ----
    qrots0 = [ac.tile([128, QB], F32, tag="ac", name=f"qrot0{q}")
              for q in range(2)]
    rope_half(0, 0, qraw0, qrots0[0])
    rope_half(0, 1, qraw0, qrots0[1])
    pend0 = attn(1, 0)
    attn(0, 0, deferred_norms=pend0)
    fire_ag(0)

    # ---- Q projection + RoPE + full attention, per m-tile pair ----
    for hp in range(1, NMT):
        wqm = wqp.tile([128, NET * 128], BF16, tag="wm")
        load_w_cols(wqm, hp)
        qps = [ps.tile([128, QB], F32, tag=f"qp{q}", name=f"qps{q}")
               for q in range(2)]
        qraw = ra.tile([128, 1024], BF16, tag="ra")
        qrots = [ac.tile([128, QB], F32, tag="ac", name=f"qrot{q}")
                 for q in range(2)]
        for i in range(NET):
            nc.tensor.matmul(
                qps[0][:, :], lhsT=wqm[:, i * 128:(i + 1) * 128],
                rhs=xsb[:, i * 1024:i * 1024 + QB],
                start=(i == 0), stop=(i == NET - 1),
            )
        with tc.high_priority():
            nc.vector.tensor_copy(qraw[:, 0:QB], qps[0][:, :])
        for i in range(NET):
            nc.tensor.matmul(
                qps[1][:, :], lhsT=wqm[:, i * 128:(i + 1) * 128],
                rhs=xsb[:, i * 1024 + QB:i * 1024 + 2 * QB],
                start=(i == 0), stop=(i == NET - 1),
            )
            if i == 5:
                rope_half(hp, 0, qraw, qrots[0])
        # only the PSUM->SBUF copies are priority-boosted (they gate the rope
        # -> scores chain and must not queue behind bulk DVE/ACT work); PE
        # attention ops keep natural order so the static schedule interleaves
        # them with the next pair's projection stream
        with tc.high_priority():
            nc.vector.tensor_copy(qraw[:, QB:2 * QB], qps[1][:, :])
        rope_half(hp, 1, qraw, qrots[1])
        pend = attn(1, hp)
        if hp == NMT - 1:
            # the qh1 AG needs its norms emitted before the trigger
            for t in pend:
                norm_post(*t)
            pend = None
            fire_ag(8)      # pair-8 q-half-1 rows, as soon as they land
        attn(0, hp, deferred_norms=pend)
        fire_ag(9 if hp == NMT - 1 else hp)
        if hp == NMT - 2:
            # scheduler fence BEFORE the last pair: dense work may interleave
            # with pair-8's attention tail, but cannot be hoisted ahead of the
            # earlier projection stream / AG triggers
            tc.no_sync_barrier()

    # free x^T/W_q SBUF; W_dense loads stream in under the attention tail
    rt.release()
    ra.release()
    wqp.release()
    xp.release()
    wdp = tc.alloc_tile_pool(name="wdp", bufs=1, side="right")
    wdsbB = wdp.tile([128, (NET - NWA) * ESH], BF16, tag="wdsbB")
    for i in range(NET - NWA):
        # scalar (ACT) HWDGE ring: keeps these 7MB of loads off the sync
        # ring so the dense gather DMAs are not queued behind them
        nc.scalar.dma_start(wdsbB[:, i * ESH:(i + 1) * ESH],
                            wdT[(NWA + i) * 128:(NWA + i + 1) * 128, :])

    # =========== dense: out^T[e_shard, q] = W_d^T[dpad, e].T @ attn^T ===========
    # wdT rows are host-permuted to chain order (pair-major over (t, r)),
    # so chain position p contracts global k-tile 9r+t.
    # The projection PSUM (ps) frees as soon as pair-8's projection is copied
    # out, so two dense chains (dpe) can run in the ACT-bound attention tail;
    # the remaining six (dp) start once the attention PSUM pools release.
    ps.release()
    dpe = tc.alloc_tile_pool(name="dpe", bufs=1, space="PSUM")
    gp0 = tc.alloc_tile_pool(name="gp0", bufs=1, side="right")
    op = tc.alloc_tile_pool(name="op", bufs=2, side="right")

    def gather_src(qh, t, r):
        row = t * 256 + qh * 128
        seg = next(s for s, (lo, hi) in enumerate(SEGS) if lo <= row < hi)
        lo, hi = SEGS[seg]
        srow = r * (hi - lo) + (row - lo)
        return ag_out[seg][srow:srow + 128, :]

    # qh0's gather buffer fits alongside the still-live es pool; qh1's is
    # allocated after the attention pools release
    gath = [gp0.tile([128, NET * QB], BF16, tag="gath0", name="gath0"), None]

    def emit_gathers(qh):
        # per-tile gather DMAs in chain order: chain MMs wait only on their
        # own tile, so the i-minor chains start on the first landed tile
        for p, (t, r) in enumerate(CHAIN):
            nc.sync.dma_start(gath[qh][:, p * QB:(p + 1) * QB],
                              gather_src(qh, t, r))

    emit_gathers(0)

    def dense_chain(dtile, qh, m, rows=128):
        for i in range(NET):
            wds, ii = (wdsbA, i) if i < NWA else (wdsbB, i - NWA)
            nc.tensor.matmul(
                dtile[0:rows, :],
                lhsT=wds[:, ii * ESH + m * 128:ii * ESH + m * 128 + rows],
                rhs=gath[qh][:, i * QB:(i + 1) * QB],
                start=(i == 0), stop=(i == NET - 1),
            )

    def evac(dtile, qh, m, rows=128):
        osb = op.tile([128, QB], F32, tag="op")
        nc.scalar.copy(osb[0:rows, :], dtile[0:rows, :])
        nc.sync.dma_start(out[m * 128:m * 128 + rows, qh * QB:(qh + 1) * QB],
                          osb[0:rows, :])

    # early chains m=0,1 of q-half 0 -- fill the attention-tail PE bubbles
    dpssE = [dpe.tile([128, QB], F32, tag=f"dpsE{m}", name=f"dpsE{m}")
             for m in range(2)]
    for m in range(2):
        dense_chain(dpssE[m], 0, m)

    ac.release()
    sc.release()
    nra.release()
    nr.release()
    ex.release()
    dp = tc.alloc_tile_pool(name="dp", bufs=1, space="PSUM")
    gp1 = tc.alloc_tile_pool(name="gp1", bufs=1, side="right")
    gath[1] = gp1.tile([128, NET * QB], BF16, tag="gath1", name="gath1")
    emit_gathers(1)

    # i-minor dense: parallel accumulation chains (one PSUM bank each) so the
    # first gather tiles feed all chains and the DMA stays ahead of the PE
    for qh in range(2):
        if qh == 0:
            dpss = dpssE + [dp.tile([128, QB], F32, tag=f"dps{m}",
                                    name=f"dps{m}") for m in range(2, 8)]
            for m in range(2, 8):
                dense_chain(dpss[m], 0, m)
        else:
            dpss = [(dpe if m < 2 else dp).tile(
                [128, QB], F32, tag=f"dps{'E' if m < 2 else ''}{m % 8 if m >= 2 else m}",
                name=f"q1dps{m}") for m in range(8)]
            for m in range(8):
                dense_chain(dpss[m], 1, m)
        for m in range(8):
            evac(dpss[m], qh, m)
        dps = dp.tile([128, QB], F32, tag="dps2", name=f"rag{qh}")
        dense_chain(dps, qh, 8, rows=112)
        evac(dps, qh, 8, rows=112)

    dp.release()
    dpe.release()
    gp1.release()
    op.release()
    gp0.release()
    wdp.release()
    wdpa.release()
    pers.release()
    dram.release()


def build():
    if "nc" in _CACHE:
        return _CACHE["nc"]
    nc = bacc.Bacc("TRN2", target_bir_lowering=False, debug=False,
                   num_devices=NCORES)
    io = {
        "xT": nc.dram_tensor("xT", [D, L], BF16, kind="ExternalInput").ap(),
        "wq3": nc.dram_tensor("wq3", [128, (NMT + 1) * NET * 128], BF16,
                              kind="ExternalInput").ap(),
        "wdT": nc.dram_tensor("wdT", [DPAD, ESH], BF16, kind="ExternalInput").ap(),
        "cosT": nc.dram_tensor("cosT", [128, L], BF16, kind="ExternalInput").ap(),
        "sinT": nc.dram_tensor("sinT", [128, L], BF16, kind="ExternalInput").ap(),
        "rot": nc.dram_tensor("rot", [128, 128], BF16, kind="ExternalInput").ap(),
        "eye": nc.dram_tensor("eye", [128, 128], BF16, kind="ExternalInput").ap(),
        "masks": nc.dram_tensor("masks", [128, 128], BF16,
                                kind="ExternalInput").ap(),
        "ones64": nc.dram_tensor("ones64", [1, 64], BF16,
                                 kind="ExternalInput").ap(),
        "out": nc.dram_tensor("out", [ESH, L], F32, kind="ExternalOutput").ap(),
    }
    with tile.TileContext(nc) as tc:
        _emit(tc, nc, io)
    nc.compile()
    _CACHE["nc"] = nc
    return nc


def make_in_maps(hidden_states, W_qkv, W_dense):
    bf = ml_dtypes.bfloat16
    x = np.asarray(hidden_states, np.float32)
    Wqkv = np.asarray(W_qkv, np.float32)
    Wd = np.asarray(W_dense, np.float32)

    # rope tables, transposed [64, L], replicated to both 64-row halves
    inv = 1.0 / (10000.0 ** (np.arange(0, DKV, 2, dtype=np.float32) / DKV))
    t = np.arange(L, dtype=np.float32)
    freqs = np.outer(t, inv)
    emb = np.concatenate([freqs, freqs], axis=1)          # [L, 64]
    cosT = np.tile(np.cos(emb).T, (2, 1)).astype(bf)      # [128, L]
    sinT = np.tile(np.sin(emb).T, (2, 1)).astype(bf)

    # rotate_half as a matmul: qrot = R1 @ q; lhsT = R1^T; 2-head block diagonal
    R1 = np.zeros((DKV, DKV), np.float32)
    for i in range(32):
        R1[i, i + 32] = -1.0
        R1[i + 32, i] = 1.0
    R2 = np.zeros((128, 128), np.float32)
    R2[:64, :64] = R1
    R2[64:, 64:] = R1
    rot = R2.T.copy().astype(bf)

    eye = np.eye(128, dtype=np.float32).astype(bf)

    ones64 = np.ones((1, 64), np.float32).astype(bf)

    # triangular causal mask for the 128x128 diagonal block
    kk = np.arange(128)[:, None]
    qq = np.arange(128)[None, :]
    masks = (kk <= qq).astype(np.float32).astype(bf)

    # padded dense weights: W_d^T with 64 zero rows appended (pad head),
    # row-tiles permuted to the dense chain order (pair-major over (t, r))
    wdT_full = np.concatenate([Wd.T, np.zeros((DPAD - D, D), np.float32)], axis=0)
    wdT_full = wdT_full.reshape(NET, 128, D)
    perm = [NMT * r + t for (t, r) in CHAIN]
    wdT_full = np.ascontiguousarray(wdT_full[perm]).reshape(DPAD, D).astype(bf)

    in_maps = []
    for c in range(NCORES):
        gg, j = divmod(c, GSZ)
        h0 = HPC * j
        nh = HPC if j < GSZ - 1 else H - HPC * (GSZ - 1)  # 18,18,18,17
        WqT = np.zeros((D, DLOC), np.float32)
        WqT[:, :nh * DKV] = Wqkv[DKV * h0:DKV * (h0 + nh)].T
        # pretile [D, 1280] -> [128, 10*36*128]: slab m holds k-tile i at
        # cols (m*36+i)*128, rows zero-padded to 4608
        Wcat = np.concatenate([WqT, Wqkv[H * DKV:].T], axis=1)   # [D, 1280]
        Wp = np.zeros((NET * 128, (NMT + 1) * 128), np.float32)
        Wp[:D] = Wcat
        wq3 = np.ascontiguousarray(
            Wp.reshape(NET, 128, NMT + 1, 128).transpose(1, 2, 0, 3)
            .reshape(128, (NMT + 1) * NET * 128)).astype(bf)
        in_maps.append({
            "xT": np.ascontiguousarray(x[gg].T).astype(bf),
            "wq3": wq3,
            "wdT": np.ascontiguousarray(wdT_full[:, ESH * j:ESH * (j + 1)]),
            "cosT": cosT, "sinT": sinT, "rot": rot, "eye": eye,
            "masks": masks, "ones64": ones64,
        })
    return in_maps


def assemble(results):
    out = np.empty((N, L, D), np.float32)
    for c in range(NCORES):
        gg, j = divmod(c, GSZ)
        out[gg, :, ESH * j:ESH * (j + 1)] = results[c]["out"].T
    return out


def kernel(hidden_states, W_qkv, W_dense):
    nc = build()
    in_maps = make_in_maps(hidden_states, W_qkv, W_dense)
    res = run_bass_kernel_spmd(nc, in_maps, core_ids=list(range(NCORES)))
    return assemble(res.results)


if __name__ == "__main__":
    import reference
    inputs = reference.setup_inputs()
    out = kernel(**{k: np.asarray(v) for k, v in inputs.items()})
    print("out", out.shape, out.dtype)
